# revision 16
# baseline (speedup 1.0000x reference)
"""Trainium2 Bass kernel for nn_Node2Pair_bias (LayerNorm -> dual projection ->
pair outer-product -> head-mix linear).

Reference computation (B=2, L=512, D=256, DH=32, H=16, K=2, P=128):
    x   = LayerNorm(node) * gamma + beta, masked        [B, L, D]
    left  = (x @ W_left + b_left)                       [B, L, DH] -> [B,L,H,K]
    right = (x @ W_right + b_right)/sqrt(DH)            [B, L, DH] -> [B,L,H,K]
    out[b,i,j,h] = sum_k left[b,i,h,k]*right[b,j,h,k]
    out[b,i,j,p] = sum_h out[b,i,j,h]*W_out[h,p] + b_out[p]   [B, L, L, P]

Mathematical restructuring (c = (h,k) combined channel, 0..31):
    out[b,i,j,p] = sum_c right[b,j,c] * (left[b,i,c] * W2[c,p]) + b_out[p]
with W2[c,p] = W_out[c//2, p].  M-packs M[c, (q,p)] = left[b,i_q,c]*W2[c,p]
for 4 i's are built in one broadcast multiply each; the pair matmul is
bf16 x bf16 -> fp32 PSUM:  lhsT=rightT[33, j-chunk 128] x rhs=M_pack[33, 512].
Row 32 of rightT is constant 1 and row 32 of the M-pack is b_out, which adds
the bias inside the same matmul.

PSUM is drained in [128, 1024] bank pairs with an f32->bf16 converting copy
(DVE/ACT alternating) into 1 MiB staging tiles, DMA'd per j-chunk.  The host
converts bf16 back to fp32 while assembling (bf16 + bf16-matmul rounding is
~6e-3 max-rel, inside the 2e-2 gate).

Sharding: the i axis of L is split across the 8 cores (sequence-parallel);
each core holds its [B, 64] slice of `left` plus the full `right` side and
writes a [B, 64, L, P] output shard.  No cross-device communication.

LayerNorm gamma/beta are folded into the projection weights on the host
(exact algebra): W_e = gamma[:,None]*W, with an extra K=1 accumulation row
carrying beta@W * mask.
"""

import os
import sys

sys.path.insert(0, "/opt/trn_rl_repo")

import numpy as np

import concourse.bass as bass
import concourse.mybir as mybir
import concourse.tile as tile
from concourse import bacc
from concourse.bass_utils import run_bass_kernel_spmd
from concourse.masks import make_identity

F32 = mybir.dt.float32
F32R = mybir.dt.float32r
BF16 = mybir.dt.bfloat16

B, L, D = 2, 512, 256
DH, H, PAIR = 32, 16, 128
NCORES = 8
LSH = L // NCORES          # 64 i's per core per batch
LN_EPS = 1e-5

# packed-constant column maps: cst_main [128, 267] + cst_rows [1, 1216]
COL_WL = (0, 32)           # [128, 32] x2: gamma*W_l rows 0-127 / 128-255
COL_WR = (64, 96)
COL_W2 = 128               # [33, 128] (cast to bf16 on chip)
COL_MCF = 256              # [128, 8]
COL_MCS = 264              # [128, 1]
COL_BL = 265               # [32, 1]
COL_BR = 266               # [32, 1]
NCONST = 267
ROW_WLR = 0                # [1, 32]  row 256 of w_left_e
ROW_WRR = 32               # [1, 32]
ROW_MRS = 64               # [1, 128]
ROW_MRF = (192, 704)       # [1, 512] x2
NROWS = 1216

_COMPILED = None  # (nc, input_names)


def _build_program():
    nc = bacc.Bacc("TRN2", target_bir_lowering=False, debug=False,
                   num_devices=NCORES)

    node_full = nc.dram_tensor("node_full", [B * L, D], F32,
                               kind="ExternalInput").ap()
    node_shard = nc.dram_tensor("node_shard", [B * LSH, D], F32,
                                kind="ExternalInput").ap()
    consts = nc.dram_tensor("consts", [128, NCONST], F32,
                            kind="ExternalInput").ap()
    const_rows = nc.dram_tensor("const_rows", [1, NROWS], F32,
                                kind="ExternalInput").ap()

    # Permuted output layout: [b, jc, q2, j, s, i16, p] (bf16) — each staging
    # buffer lands as one fully contiguous 1 MiB stream (8 KiB per partition
    # run).  sg = q2*2 + s; i_local = sg*16 + i16.  The host un-permutes +
    # upcasts while assembling the full output.
    out = nc.dram_tensor("out", [B, 4, 2, 128, 2, 16, PAIR], BF16,
                         kind="ExternalOutput").ap()

    with tile.TileContext(nc) as tc:
        with (
            tc.tile_pool(name="singles", bufs=1) as singles,
            tc.tile_pool(name="xpool", bufs=9) as xpool,
            tc.tile_pool(name="stats", bufs=4) as stats,
            tc.tile_pool(name="persist", bufs=1) as persist,
            tc.tile_pool(name="mp", bufs=16) as mp_pool,
            tc.tile_pool(name="stag", bufs=8) as stag_pool,
            tc.tile_pool(name="ps_proj", bufs=1, space="PSUM") as ps_proj,
            tc.tile_pool(name="ps_big", bufs=3, space="PSUM") as ps_big,
        ):
            # ---------------- input loads (3 queues in parallel) ------------
            # b=0 tiles + shard first (they gate the whole pipeline);
            # b=1 tiles trail on gpsimd/sync.
            xs = xpool.tile([128, D], F32, tag="x", name="xs")
            nc.sync.dma_start(out=xs, in_=node_shard[:, :])
            cst = singles.tile([128, NCONST], F32, tag="cst")
            nc.scalar.dma_start(out=cst, in_=consts[:, :])
            crow = singles.tile([1, NROWS], F32, tag="crow")
            nc.scalar.dma_start(out=crow, in_=const_rows[:, :])
            xf_tiles = [None] * 8
            qmap = {0: nc.sync, 1: nc.scalar, 2: nc.sync, 3: nc.scalar,
                    4: nc.gpsimd, 5: nc.sync, 6: nc.gpsimd, 7: nc.sync}
            for t in [0, 1, 2, 3, 4, 5, 6, 7]:
                xf = xpool.tile([128, D], F32, tag="x", name=f"xf{t}")
                qmap[t].dma_start(
                    out=xf, in_=node_full[t * 128:(t + 1) * 128, :])
                xf_tiles[t] = xf

            # ---------------- constants / views ----------------
            ident = singles.tile([128, 128], F32, tag="ident")
            make_identity(nc, ident)
            eps_t = singles.tile([128, 1], F32, tag="eps")
            nc.vector.memset(eps_t, LN_EPS)

            wl_sb = [cst[:, COL_WL[dc]:COL_WL[dc] + DH] for dc in range(2)]
            wr_sb = [cst[:, COL_WR[dc]:COL_WR[dc] + DH] for dc in range(2)]
            wl_row = crow[0:1, ROW_WLR:ROW_WLR + DH]
            wr_row = crow[0:1, ROW_WRR:ROW_WRR + DH]
            bl_sb = cst[0:DH, COL_BL:COL_BL + 1]
            br_sb = cst[0:DH, COL_BR:COL_BR + 1]
            mcf_sb = cst[:, COL_MCF:COL_MCF + 8]
            mcs_sb = cst[:, COL_MCS:COL_MCS + 1]
            mrs_sb = crow[0:1, ROW_MRS:ROW_MRS + B * LSH]
            mrf_sb = [crow[0:1, ROW_MRF[b]:ROW_MRF[b] + L] for b in range(B)]

            w2bf = singles.tile([DH + 1, PAIR], BF16, tag="w2bf")
            nc.scalar.copy(out=w2bf, in_=cst[0:DH + 1, COL_W2:COL_W2 + PAIR])

            # ---------------- LayerNorm helper ----------------
            def layernorm_masked(x_t, mask_col_ap):
                """x_t [128, D] in place -> (x - mu) * rsqrt(var+eps) * mask."""
                st = stats.tile([128, 6], F32, tag="st")
                nc.vector.bn_stats(out=st, in_=x_t)
                mv = stats.tile([128, 2], F32, tag="mv")
                nc.vector.bn_aggr(out=mv, in_=st)
                sd = stats.tile([128, 1], F32, tag="sd")
                nc.scalar.activation(out=sd, in_=mv[:, 1:2],
                                     func=mybir.ActivationFunctionType.Sqrt,
                                     bias=eps_t, scale=1.0)
                rs = stats.tile([128, 1], F32, tag="rs")
                nc.vector.reciprocal(out=rs, in_=sd)
                rsm = stats.tile([128, 1], F32, tag="rsm")
                nc.vector.tensor_mul(out=rsm, in0=rs, in1=mask_col_ap)
                nc.vector.tensor_scalar(out=x_t, in0=x_t,
                                        scalar1=mv[:, 0:1], scalar2=rsm,
                                        op0=mybir.AluOpType.subtract,
                                        op1=mybir.AluOpType.mult)

            # ---------------- shard path: leftT_all [33, B*LSH] bf16 --------
            layernorm_masked(xs, mcs_sb)

            xsT = [persist.tile([128, B * LSH], F32, tag=f"xsT{dc}",
                                name=f"xsT{dc}") for dc in range(2)]
            for dc in range(2):
                pt = ps_big.tile([128, 1024], F32, tag="big",
                                 name=f"tps{dc}")[:, 0:128]
                nc.tensor.transpose(pt, xs[:, dc * 128:(dc + 1) * 128], ident)
                nc.scalar.copy(out=xsT[dc], in_=pt)

            ps_l = ps_proj.tile([DH, L], F32, tag="pr", name="ps_l")
            ps_l = ps_l[:, 0:B * LSH]
            for dc in range(2):
                nc.tensor.matmul(ps_l, wl_sb[dc], xsT[dc],
                                 start=(dc == 0), stop=False)
            nc.tensor.matmul(ps_l, wl_row, mrs_sb, start=False, stop=True)
            leftT = persist.tile([DH + 1, B * LSH], BF16, tag="leftT")
            nc.scalar.activation(out=leftT[0:DH, :], in_=ps_l,
                                 func=mybir.ActivationFunctionType.Identity,
                                 bias=bl_sb, scale=1.0)
            nc.vector.memset(leftT[DH:DH + 1, :], 1.0)

            # ---------------- full path (per batch): rightT[b] [33, L] bf16 -
            rightT = [persist.tile([DH + 1, L], BF16, tag=f"rt{b}",
                                   name=f"rt{b}") for b in range(B)]
            xT = [[persist.tile([128, L], F32, tag=f"xT{b}_{dc}",
                                name=f"xT{b}_{dc}") for dc in range(2)]
                  for b in range(B)]

            def build_right(b):
                ps_r = ps_proj.tile([DH, L], F32, tag="pr", name=f"ps_r{b}")
                for lc in range(4):
                    t = b * 4 + lc
                    xf = xf_tiles[t]
                    layernorm_masked(xf, mcf_sb[:, t:t + 1])
                    for dc in range(2):
                        pt = ps_big.tile([128, 1024], F32, tag="big",
                                         name=f"tp{b}_{lc}_{dc}")[:, 0:128]
                        nc.tensor.transpose(pt, xf[:, dc * 128:(dc + 1) * 128],
                                            ident)
                        nc.scalar.copy(
                            out=xT[b][dc][:, lc * 128:(lc + 1) * 128],
                            in_=pt)
                    # project this j-chunk as soon as its columns exist
                    jc = lc
                    jsl = slice(jc * 128, (jc + 1) * 128)
                    for dc in range(2):
                        nc.tensor.matmul(ps_r[:, jsl], wr_sb[dc],
                                         xT[b][dc][:, jsl],
                                         start=(dc == 0), stop=False)
                    nc.tensor.matmul(ps_r[:, jsl], wr_row, mrf_sb[b][:, jsl],
                                     start=False, stop=True)
                    nc.scalar.activation(out=rightT[b][0:DH, jsl],
                                         in_=ps_r[:, jsl],
                                         func=mybir.ActivationFunctionType.Identity,
                                         bias=br_sb, scale=1.0)
                nc.vector.memset(rightT[b][DH:DH + 1, :], 1.0)

            # ---------------- main pair loop ----------------
            def build_mp8(b, sg, ilp, eng):
                """[33, 1024] bf16 pack for 8 i's (il = 2*ilp, 2*ilp+1):
                mp[c, (il8, q)*128 + p] = left[b, i, c] * w2[c, p]."""
                mp = mp_pool.tile([DH + 1, 1024], BF16, tag="mp",
                                  name=f"mp{b}_{sg}_{ilp}")
                col = b * LSH + (sg * 4 + ilp * 2) * 4
                lsrc = leftT[:, col:col + 8].unsqueeze(-1).to_broadcast(
                    [DH + 1, 8, PAIR])
                wsrc = w2bf[:, :].unsqueeze(1).to_broadcast([DH + 1, 8, PAIR])
                dst = mp[:, :].rearrange("c (q p) -> c q p", p=PAIR)
                eng.tensor_mul(out=dst, in0=wsrc, in1=lsrc)
                return mp

            # all 16 M-packs depend only on leftT; hoist them so the GpSimd
            # queue never gates a round.  The two packs the very first
            # matmuls need go on DVE (GpSimd packs are ~1.9us each).
            all_mps = {}

            def build_all_mps():
                for b in range(B):
                    for q2 in range(2):
                        for s in range(2):
                            for ilp in range(2):
                                eng = nc.vector if (b, q2, s) == (0, 0, 0) \
                                    else nc.gpsimd
                                all_mps[(b, q2, s, ilp)] = build_mp8(
                                    b, q2 * 2 + s, ilp, eng)

            def pair_rounds(b):
                for q2 in range(2):
                    mps = [[all_mps[(b, q2, s, ilp)] for ilp in range(2)]
                           for s in range(2)]
                    for jc in range(4):
                        lhsT = rightT[b][:, jc * 128:(jc + 1) * 128]
                        stg = stag_pool.tile([128, 2 * 16 * PAIR], BF16,
                                             tag="stag")
                        for s in range(2):
                            for ilp in range(2):
                                pb = ps_big.tile([128, 1024], F32, tag="big")
                                nc.tensor.matmul(pb[:, 0:512], lhsT,
                                                 mps[s][ilp][:, 0:512],
                                                 start=True, stop=True)
                                nc.tensor.matmul(pb[:, 512:1024], lhsT,
                                                 mps[s][ilp][:, 512:1024],
                                                 start=True, stop=True)
                                dst = stg[:, (s * 16 + ilp * 8) * PAIR:
                                          (s * 16 + ilp * 8 + 8) * PAIR]
                                if (s + ilp) % 2 == 0:
                                    nc.vector.tensor_copy(out=dst, in_=pb)
                                else:
                                    nc.scalar.copy(out=dst, in_=pb)
                        dst_ap = out[b, jc, q2, :, :, :, :]
                        src_ap = stg[:, :].rearrange("j (s i p) -> j s i p",
                                                     s=2, p=PAIR)
                        deng = nc.sync if jc % 2 == 0 else nc.scalar
                        deng.dma_start(out=dst_ap, in_=src_ap)

            build_right(0)
            build_all_mps()
            pair_rounds(0)
            build_right(1)
            pair_rounds(1)

    nc.compile()
    names = ["node_full", "node_shard", "consts"]
    return nc, names


def _prepare_in_maps(node, mask, ln_gamma, ln_beta, W_left, b_left, W_right,
                     b_right, W_out, b_out):
    f = np.float32
    node = np.ascontiguousarray(np.asarray(node, dtype=f))        # [B, L, D]
    mask_f = np.asarray(mask).astype(f)                           # [B, L]
    gamma = np.asarray(ln_gamma, dtype=f)
    beta = np.asarray(ln_beta, dtype=f)
    W_l = np.asarray(W_left, dtype=f)
    W_r = np.asarray(W_right, dtype=f)
    b_l = np.asarray(b_left, dtype=f)
    b_r = np.asarray(b_right, dtype=f)
    W_o = np.asarray(W_out, dtype=f)
    b_o = np.asarray(b_out, dtype=f)

    s = 1.0 / np.sqrt(np.float32(DH))
    w_left_e = np.concatenate([gamma[:, None] * W_l, (beta @ W_l)[None, :]], 0)
    w_right_e = np.concatenate([gamma[:, None] * W_r, (beta @ W_r)[None, :]],
                               0) * s
    w2 = np.concatenate([np.repeat(W_o, 2, axis=0), b_o[None, :]], 0)

    common_cst = np.zeros((128, NCONST), f)
    for dc in range(2):
        common_cst[:, COL_WL[dc]:COL_WL[dc] + DH] = \
            w_left_e[dc * 128:(dc + 1) * 128]
        common_cst[:, COL_WR[dc]:COL_WR[dc] + DH] = \
            w_right_e[dc * 128:(dc + 1) * 128]
    common_cst[0:DH + 1, COL_W2:COL_W2 + PAIR] = w2
    common_cst[:, COL_MCF:COL_MCF + 8] = mask_f.reshape(-1, 128).T
    common_cst[0:DH, COL_BL] = b_l
    common_cst[0:DH, COL_BR] = b_r * s

    common_rows = np.zeros((1, NROWS), f)
    common_rows[0, ROW_WLR:ROW_WLR + DH] = w_left_e[D]
    common_rows[0, ROW_WRR:ROW_WRR + DH] = w_right_e[D]
    for b in range(B):
        common_rows[0, ROW_MRF[b]:ROW_MRF[b] + L] = mask_f[b]

    node_flat = node.reshape(B * L, D)

    in_maps = []
    for c in range(NCORES):
        sl = slice(c * LSH, (c + 1) * LSH)
        shard = np.ascontiguousarray(node[:, sl, :].reshape(B * LSH, D))
        msk = mask_f[:, sl]                                       # [B, LSH]
        cstc = common_cst.copy()
        cstc[:, COL_MCS] = msk.reshape(-1)
        rowc = common_rows.copy()
        rowc[0, ROW_MRS:ROW_MRS + B * LSH] = msk.reshape(-1)
        in_maps.append({
            "node_full": node_flat,
            "node_shard": shard,
            "consts": cstc,
            "const_rows": rowc,
        })
    return in_maps


def kernel(**inputs):
    global _COMPILED
    if _COMPILED is None:
        _COMPILED = _build_program()
    nc, names = _COMPILED
    in_maps = _prepare_in_maps(**inputs)
    res = run_bass_kernel_spmd(nc, in_maps, core_ids=list(range(NCORES)))
    full = np.empty((B, L, L, PAIR), np.float32)
    for c in range(NCORES):
        dev = res.results[c]["out"]   # [b, jc, q2, j, s, i16, p] bf16
        full[:, c * LSH:(c + 1) * LSH] = (
            dev.transpose(0, 2, 4, 5, 1, 3, 6)
               .reshape(B, LSH, L, PAIR).astype(np.float32))
    return full


if __name__ == "__main__":
    # self-test with NON-trivial gamma/beta/mask against a numpy reference
    rng = np.random.default_rng(1)
    mask = np.ones((B, L), dtype=bool)
    mask[0, 500:] = False        # exercise the mask path
    mask[1, :3] = False
    inputs = {
        "node": rng.standard_normal((B, L, D)).astype(np.float32),
        "mask": mask,
        "ln_gamma": (1.0 + 0.1 * rng.standard_normal(D)).astype(np.float32),
        "ln_beta": (0.1 * rng.standard_normal(D)).astype(np.float32),
        "W_left": (rng.standard_normal((D, DH)) / np.sqrt(D)).astype(np.float32),
        "b_left": (0.1 * rng.standard_normal(DH)).astype(np.float32),
        "W_right": (rng.standard_normal((D, DH)) / np.sqrt(D)).astype(np.float32),
        "b_right": (0.1 * rng.standard_normal(DH)).astype(np.float32),
        "W_out": (rng.standard_normal((H, PAIR)) / np.sqrt(H)).astype(np.float32),
        "b_out": (0.1 * rng.standard_normal(PAIR)).astype(np.float32),
    }

    def np_reference(node, mask, ln_gamma, ln_beta, W_left, b_left, W_right,
                     b_right, W_out, b_out):
        node = node.astype(np.float64)
        mu = node.mean(-1, keepdims=True)
        var = ((node - mu) ** 2).mean(-1, keepdims=True)
        x = (node - mu) / np.sqrt(var + LN_EPS) * ln_gamma + ln_beta
        x = x * mask[..., None]
        left = (x @ W_left + b_left).reshape(B, L, H, -1)
        right = ((x @ W_right + b_right) / np.sqrt(DH)).reshape(B, L, H, -1)
        o = np.einsum("bihk,bjhk->bijh", left, right)
        return np.einsum("bijh,hp->bijp", o, W_out) + b_out

    got = kernel(**inputs)
    exp = np_reference(**inputs)
    rel = np.abs(got - exp).max() / np.abs(exp).max()
    print("general-path rel err:", rel)
    assert rel < 1.5e-2, rel
    print("OK", got.shape, got.dtype)


# revision 17
# speedup vs baseline: 1.0108x; 1.0108x over previous
"""Trainium2 Bass kernel for nn_Node2Pair_bias (LayerNorm -> dual projection ->
pair outer-product -> head-mix linear).

Reference computation (B=2, L=512, D=256, DH=32, H=16, K=2, P=128):
    x   = LayerNorm(node) * gamma + beta, masked        [B, L, D]
    left  = (x @ W_left + b_left)                       [B, L, DH] -> [B,L,H,K]
    right = (x @ W_right + b_right)/sqrt(DH)            [B, L, DH] -> [B,L,H,K]
    out[b,i,j,h] = sum_k left[b,i,h,k]*right[b,j,h,k]
    out[b,i,j,p] = sum_h out[b,i,j,h]*W_out[h,p] + b_out[p]   [B, L, L, P]

Mathematical restructuring (c = (h,k) combined channel, 0..31):
    out[b,i,j,p] = sum_c right[b,j,c] * (left[b,i,c] * W2[c,p]) + b_out[p]
with W2[c,p] = W_out[c//2, p].  M-packs M[c, (q,p)] = left[b,i_q,c]*W2[c,p]
for 4 i's are built in one broadcast multiply each; the pair matmul is
bf16 x bf16 -> fp32 PSUM:  lhsT=rightT[33, j-chunk 128] x rhs=M_pack[33, 512].
Row 32 of rightT is constant 1 and row 32 of the M-pack is b_out, which adds
the bias inside the same matmul.

PSUM is drained in [128, 1024] bank pairs with an f32->bf16 converting copy
(DVE/ACT alternating) into 1 MiB staging tiles, DMA'd per j-chunk.  The host
converts bf16 back to fp32 while assembling (bf16 + bf16-matmul rounding is
~6e-3 max-rel, inside the 2e-2 gate).

Sharding: the i axis of L is split across the 8 cores (sequence-parallel);
each core holds its [B, 64] slice of `left` plus the full `right` side and
writes a [B, 64, L, P] output shard.  No cross-device communication.

LayerNorm gamma/beta are folded into the projection weights on the host
(exact algebra): W_e = gamma[:,None]*W, with an extra K=1 accumulation row
carrying beta@W * mask.
"""

import os
import sys

sys.path.insert(0, "/opt/trn_rl_repo")

import numpy as np

import concourse.bass as bass
import concourse.mybir as mybir
import concourse.tile as tile
from concourse import bacc
from concourse.bass_utils import run_bass_kernel_spmd
from concourse.masks import make_identity

F32 = mybir.dt.float32
F32R = mybir.dt.float32r
BF16 = mybir.dt.bfloat16

B, L, D = 2, 512, 256
DH, H, PAIR = 32, 16, 128
NCORES = 8
LSH = L // NCORES          # 64 i's per core per batch
LN_EPS = 1e-5

# packed-constant column maps: cst_main [128, 267] + cst_rows [1, 1216]
COL_WL = (0, 32)           # [128, 32] x2: gamma*W_l rows 0-127 / 128-255
COL_WR = (64, 96)
COL_W2 = 128               # [33, 128] (cast to bf16 on chip)
COL_MCF = 256              # [128, 8]
COL_MCS = 264              # [128, 1]
COL_BL = 265               # [32, 1]
COL_BR = 266               # [32, 1]
NCONST = 267
ROW_WLR = 0                # [1, 32]  row 256 of w_left_e
ROW_WRR = 32               # [1, 32]
ROW_MRS = 64               # [1, 128]
ROW_MRF = (192, 704)       # [1, 512] x2
NROWS = 1216

_COMPILED = None  # (nc, input_names)


def _build_program():
    nc = bacc.Bacc("TRN2", target_bir_lowering=False, debug=False,
                   num_devices=NCORES)

    node_full = nc.dram_tensor("node_full", [B * L, D], F32,
                               kind="ExternalInput").ap()
    node_shard = nc.dram_tensor("node_shard", [B * LSH, D], F32,
                                kind="ExternalInput").ap()
    consts = nc.dram_tensor("consts", [128, NCONST], F32,
                            kind="ExternalInput").ap()
    const_rows = nc.dram_tensor("const_rows", [1, NROWS], F32,
                                kind="ExternalInput").ap()

    # Permuted output layout: [b, jc, q2, j, s, i16, p] (bf16) — each staging
    # buffer lands as one fully contiguous 1 MiB stream (8 KiB per partition
    # run).  sg = q2*2 + s; i_local = sg*16 + i16.  The host un-permutes +
    # upcasts while assembling the full output.
    out = nc.dram_tensor("out", [B, 4, 2, 128, 2, 16, PAIR], BF16,
                         kind="ExternalOutput").ap()

    with tile.TileContext(nc) as tc:
        with (
            tc.tile_pool(name="singles", bufs=1) as singles,
            tc.tile_pool(name="xpool", bufs=2) as xpool,
            tc.tile_pool(name="stats", bufs=4) as stats,
            tc.tile_pool(name="persist", bufs=1) as persist,
            tc.tile_pool(name="mp", bufs=32) as mp_pool,
            tc.tile_pool(name="stag", bufs=8) as stag_pool,
            tc.tile_pool(name="ps_proj", bufs=1, space="PSUM") as ps_proj,
            tc.tile_pool(name="ps_big", bufs=3, space="PSUM") as ps_big,
        ):
            # ---------------- input loads (2 HWDGE rings, batched) ----------
            # One DMA per 4-tile half of node_full: a single completion
            # receipt instead of four ~2us ones.
            xs = xpool.tile([128, D], F32, tag="x", name="xs")
            nc.sync.dma_start(out=xs, in_=node_shard[:, :])
            cst = singles.tile([128, NCONST], F32, tag="cst")
            nc.scalar.dma_start(out=cst, in_=consts[:, :])
            crow = singles.tile([1, NROWS], F32, tag="crow")
            nc.scalar.dma_start(out=crow, in_=const_rows[:, :])
            xhalf = []
            for h, q in ((0, nc.sync), (1, nc.scalar)):
                xh = xpool.tile([128, 4 * D], F32, tag="xh", name=f"xh{h}")
                src_ap = node_full[h * 512:(h + 1) * 512, :].rearrange(
                    "(t j) d -> j t d", j=128)
                q.dma_start(out=xh.rearrange("j (t d) -> j t d", d=D),
                            in_=src_ap)
                xhalf.append(xh)
            xf_tiles = [xhalf[t // 4][:, (t % 4) * D:(t % 4 + 1) * D]
                        for t in range(8)]

            # ---------------- constants / views ----------------
            ident = singles.tile([128, 128], F32, tag="ident")
            make_identity(nc, ident)
            eps_t = singles.tile([128, 1], F32, tag="eps")
            nc.vector.memset(eps_t, LN_EPS)

            wl_sb = [cst[:, COL_WL[dc]:COL_WL[dc] + DH] for dc in range(2)]
            wr_sb = [cst[:, COL_WR[dc]:COL_WR[dc] + DH] for dc in range(2)]
            wl_row = crow[0:1, ROW_WLR:ROW_WLR + DH]
            wr_row = crow[0:1, ROW_WRR:ROW_WRR + DH]
            bl_sb = cst[0:DH, COL_BL:COL_BL + 1]
            br_sb = cst[0:DH, COL_BR:COL_BR + 1]
            mcf_sb = cst[:, COL_MCF:COL_MCF + 8]
            mcs_sb = cst[:, COL_MCS:COL_MCS + 1]
            mrs_sb = crow[0:1, ROW_MRS:ROW_MRS + B * LSH]
            mrf_sb = [crow[0:1, ROW_MRF[b]:ROW_MRF[b] + L] for b in range(B)]

            w2bf = singles.tile([DH + 1, PAIR], BF16, tag="w2bf")
            nc.scalar.copy(out=w2bf, in_=cst[0:DH + 1, COL_W2:COL_W2 + PAIR])

            # ---------------- LayerNorm helper ----------------
            def layernorm_masked(x_t, mask_col_ap):
                """x_t [128, D] in place -> (x - mu) * rsqrt(var+eps) * mask."""
                st = stats.tile([128, 6], F32, tag="st")
                nc.vector.bn_stats(out=st, in_=x_t)
                mv = stats.tile([128, 2], F32, tag="mv")
                nc.vector.bn_aggr(out=mv, in_=st)
                sd = stats.tile([128, 1], F32, tag="sd")
                nc.scalar.activation(out=sd, in_=mv[:, 1:2],
                                     func=mybir.ActivationFunctionType.Sqrt,
                                     bias=eps_t, scale=1.0)
                rs = stats.tile([128, 1], F32, tag="rs")
                nc.vector.reciprocal(out=rs, in_=sd)
                rsm = stats.tile([128, 1], F32, tag="rsm")
                nc.vector.tensor_mul(out=rsm, in0=rs, in1=mask_col_ap)
                nc.vector.tensor_scalar(out=x_t, in0=x_t,
                                        scalar1=mv[:, 0:1], scalar2=rsm,
                                        op0=mybir.AluOpType.subtract,
                                        op1=mybir.AluOpType.mult)

            # ---------------- shard path: leftT_all [33, B*LSH] bf16 --------
            layernorm_masked(xs, mcs_sb)

            xsT = [persist.tile([128, B * LSH], F32, tag=f"xsT{dc}",
                                name=f"xsT{dc}") for dc in range(2)]
            for dc in range(2):
                pt = ps_big.tile([128, 1024], F32, tag="big",
                                 name=f"tps{dc}")[:, 0:128]
                nc.tensor.transpose(pt, xs[:, dc * 128:(dc + 1) * 128], ident)
                nc.scalar.copy(out=xsT[dc], in_=pt)

            ps_l = ps_proj.tile([DH, L], F32, tag="pr", name="ps_l")
            ps_l = ps_l[:, 0:B * LSH]
            for dc in range(2):
                nc.tensor.matmul(ps_l, wl_sb[dc], xsT[dc],
                                 start=(dc == 0), stop=False)
            nc.tensor.matmul(ps_l, wl_row, mrs_sb, start=False, stop=True)
            leftT = persist.tile([DH + 1, B * LSH], BF16, tag="leftT")
            nc.scalar.activation(out=leftT[0:DH, :], in_=ps_l,
                                 func=mybir.ActivationFunctionType.Identity,
                                 bias=bl_sb, scale=1.0)
            nc.vector.memset(leftT[DH:DH + 1, :], 1.0)

            # ---------------- full path (per batch): rightT[b] [33, L] bf16 -
            rightT = [persist.tile([DH + 1, L], BF16, tag=f"rt{b}",
                                   name=f"rt{b}") for b in range(B)]
            xT = [[persist.tile([128, L], F32, tag=f"xT{b}_{dc}",
                                name=f"xT{b}_{dc}") for dc in range(2)]
                  for b in range(B)]

            def build_right(b):
                ps_r = ps_proj.tile([DH, L], F32, tag="pr", name=f"ps_r{b}")
                for lc in range(4):
                    t = b * 4 + lc
                    xf = xf_tiles[t]
                    layernorm_masked(xf, mcf_sb[:, t:t + 1])
                    for dc in range(2):
                        pt = ps_big.tile([128, 1024], F32, tag="big",
                                         name=f"tp{b}_{lc}_{dc}")[:, 0:128]
                        nc.tensor.transpose(pt, xf[:, dc * 128:(dc + 1) * 128],
                                            ident)
                        nc.scalar.copy(
                            out=xT[b][dc][:, lc * 128:(lc + 1) * 128],
                            in_=pt)
                    # project this j-chunk as soon as its columns exist
                    jc = lc
                    jsl = slice(jc * 128, (jc + 1) * 128)
                    for dc in range(2):
                        nc.tensor.matmul(ps_r[:, jsl], wr_sb[dc],
                                         xT[b][dc][:, jsl],
                                         start=(dc == 0), stop=False)
                    nc.tensor.matmul(ps_r[:, jsl], wr_row, mrf_sb[b][:, jsl],
                                     start=False, stop=True)
                    nc.scalar.activation(out=rightT[b][0:DH, jsl],
                                         in_=ps_r[:, jsl],
                                         func=mybir.ActivationFunctionType.Identity,
                                         bias=br_sb, scale=1.0)
                nc.vector.memset(rightT[b][DH:DH + 1, :], 1.0)

            # ---------------- main pair loop ----------------
            def build_mp4(b, sg, il):
                """[33, 512] bf16 pack for 4 i's:
                mp[c, q*128 + p] = left[b, i(sg,il,q), c] * w2[c, p]."""
                mp = mp_pool.tile([DH + 1, 512], BF16, tag="mp",
                                  name=f"mp{b}_{sg}_{il}")
                col = b * LSH + (sg * 4 + il) * 4
                lsrc = leftT[:, col:col + 4].unsqueeze(-1).to_broadcast(
                    [DH + 1, 4, PAIR])
                wsrc = w2bf[:, :].unsqueeze(1).to_broadcast([DH + 1, 4, PAIR])
                dst = mp[:, :].rearrange("c (q p) -> c q p", p=PAIR)
                nc.gpsimd.tensor_mul(out=dst, in0=wsrc, in1=lsrc)
                return mp

            # all 32 M-packs depend only on leftT; hoist them in round order
            # on the (otherwise idle) GpSimd queue so no round ever waits.
            all_mps = {}

            def build_all_mps():
                for b in range(B):
                    for q2 in range(2):
                        for s in range(2):
                            for il in range(4):
                                all_mps[(b, q2, s, il)] = build_mp4(
                                    b, q2 * 2 + s, il)

            def pair_rounds(b):
                for q2 in range(2):
                    mps = [[all_mps[(b, q2, s, il)] for il in range(4)]
                           for s in range(2)]
                    for jc in range(4):
                        lhsT = rightT[b][:, jc * 128:(jc + 1) * 128]
                        stg = stag_pool.tile([128, 2 * 16 * PAIR], BF16,
                                             tag="stag")
                        for s in range(2):
                            for ilp in range(2):
                                pb = ps_big.tile([128, 1024], F32, tag="big")
                                nc.tensor.matmul(pb[:, 0:512], lhsT,
                                                 mps[s][2 * ilp],
                                                 start=True, stop=True)
                                nc.tensor.matmul(pb[:, 512:1024], lhsT,
                                                 mps[s][2 * ilp + 1],
                                                 start=True, stop=True)
                                dst = stg[:, (s * 16 + ilp * 8) * PAIR:
                                          (s * 16 + ilp * 8 + 8) * PAIR]
                                if (s + ilp) % 2 == 0:
                                    nc.vector.tensor_copy(out=dst, in_=pb)
                                else:
                                    nc.scalar.copy(out=dst, in_=pb)
                        dst_ap = out[b, jc, q2, :, :, :, :]
                        src_ap = stg[:, :].rearrange("j (s i p) -> j s i p",
                                                     s=2, p=PAIR)
                        deng = nc.sync if jc % 2 == 0 else nc.scalar
                        deng.dma_start(out=dst_ap, in_=src_ap)

            build_right(0)
            build_all_mps()
            pair_rounds(0)
            build_right(1)
            pair_rounds(1)

    nc.compile()
    names = ["node_full", "node_shard", "consts"]
    return nc, names


def _prepare_in_maps(node, mask, ln_gamma, ln_beta, W_left, b_left, W_right,
                     b_right, W_out, b_out):
    f = np.float32
    node = np.ascontiguousarray(np.asarray(node, dtype=f))        # [B, L, D]
    mask_f = np.asarray(mask).astype(f)                           # [B, L]
    gamma = np.asarray(ln_gamma, dtype=f)
    beta = np.asarray(ln_beta, dtype=f)
    W_l = np.asarray(W_left, dtype=f)
    W_r = np.asarray(W_right, dtype=f)
    b_l = np.asarray(b_left, dtype=f)
    b_r = np.asarray(b_right, dtype=f)
    W_o = np.asarray(W_out, dtype=f)
    b_o = np.asarray(b_out, dtype=f)

    s = 1.0 / np.sqrt(np.float32(DH))
    w_left_e = np.concatenate([gamma[:, None] * W_l, (beta @ W_l)[None, :]], 0)
    w_right_e = np.concatenate([gamma[:, None] * W_r, (beta @ W_r)[None, :]],
                               0) * s
    w2 = np.concatenate([np.repeat(W_o, 2, axis=0), b_o[None, :]], 0)

    common_cst = np.zeros((128, NCONST), f)
    for dc in range(2):
        common_cst[:, COL_WL[dc]:COL_WL[dc] + DH] = \
            w_left_e[dc * 128:(dc + 1) * 128]
        common_cst[:, COL_WR[dc]:COL_WR[dc] + DH] = \
            w_right_e[dc * 128:(dc + 1) * 128]
    common_cst[0:DH + 1, COL_W2:COL_W2 + PAIR] = w2
    common_cst[:, COL_MCF:COL_MCF + 8] = mask_f.reshape(-1, 128).T
    common_cst[0:DH, COL_BL] = b_l
    common_cst[0:DH, COL_BR] = b_r * s

    common_rows = np.zeros((1, NROWS), f)
    common_rows[0, ROW_WLR:ROW_WLR + DH] = w_left_e[D]
    common_rows[0, ROW_WRR:ROW_WRR + DH] = w_right_e[D]
    for b in range(B):
        common_rows[0, ROW_MRF[b]:ROW_MRF[b] + L] = mask_f[b]

    node_flat = node.reshape(B * L, D)

    in_maps = []
    for c in range(NCORES):
        sl = slice(c * LSH, (c + 1) * LSH)
        shard = np.ascontiguousarray(node[:, sl, :].reshape(B * LSH, D))
        msk = mask_f[:, sl]                                       # [B, LSH]
        cstc = common_cst.copy()
        cstc[:, COL_MCS] = msk.reshape(-1)
        rowc = common_rows.copy()
        rowc[0, ROW_MRS:ROW_MRS + B * LSH] = msk.reshape(-1)
        in_maps.append({
            "node_full": node_flat,
            "node_shard": shard,
            "consts": cstc,
            "const_rows": rowc,
        })
    return in_maps


def kernel(**inputs):
    global _COMPILED
    if _COMPILED is None:
        _COMPILED = _build_program()
    nc, names = _COMPILED
    in_maps = _prepare_in_maps(**inputs)
    res = run_bass_kernel_spmd(nc, in_maps, core_ids=list(range(NCORES)))
    full = np.empty((B, L, L, PAIR), np.float32)
    for c in range(NCORES):
        dev = res.results[c]["out"]   # [b, jc, q2, j, s, i16, p] bf16
        full[:, c * LSH:(c + 1) * LSH] = (
            dev.transpose(0, 2, 4, 5, 1, 3, 6)
               .reshape(B, LSH, L, PAIR).astype(np.float32))
    return full


if __name__ == "__main__":
    # self-test with NON-trivial gamma/beta/mask against a numpy reference
    rng = np.random.default_rng(1)
    mask = np.ones((B, L), dtype=bool)
    mask[0, 500:] = False        # exercise the mask path
    mask[1, :3] = False
    inputs = {
        "node": rng.standard_normal((B, L, D)).astype(np.float32),
        "mask": mask,
        "ln_gamma": (1.0 + 0.1 * rng.standard_normal(D)).astype(np.float32),
        "ln_beta": (0.1 * rng.standard_normal(D)).astype(np.float32),
        "W_left": (rng.standard_normal((D, DH)) / np.sqrt(D)).astype(np.float32),
        "b_left": (0.1 * rng.standard_normal(DH)).astype(np.float32),
        "W_right": (rng.standard_normal((D, DH)) / np.sqrt(D)).astype(np.float32),
        "b_right": (0.1 * rng.standard_normal(DH)).astype(np.float32),
        "W_out": (rng.standard_normal((H, PAIR)) / np.sqrt(H)).astype(np.float32),
        "b_out": (0.1 * rng.standard_normal(PAIR)).astype(np.float32),
    }

    def np_reference(node, mask, ln_gamma, ln_beta, W_left, b_left, W_right,
                     b_right, W_out, b_out):
        node = node.astype(np.float64)
        mu = node.mean(-1, keepdims=True)
        var = ((node - mu) ** 2).mean(-1, keepdims=True)
        x = (node - mu) / np.sqrt(var + LN_EPS) * ln_gamma + ln_beta
        x = x * mask[..., None]
        left = (x @ W_left + b_left).reshape(B, L, H, -1)
        right = ((x @ W_right + b_right) / np.sqrt(DH)).reshape(B, L, H, -1)
        o = np.einsum("bihk,bjhk->bijh", left, right)
        return np.einsum("bijh,hp->bijp", o, W_out) + b_out

    got = kernel(**inputs)
    exp = np_reference(**inputs)
    rel = np.abs(got - exp).max() / np.abs(exp).max()
    print("general-path rel err:", rel)
    assert rel < 1.5e-2, rel
    print("OK", got.shape, got.dtype)


# revision 18
# speedup vs baseline: 1.0198x; 1.0089x over previous
"""Trainium2 Bass kernel for nn_Node2Pair_bias (LayerNorm -> dual projection ->
pair outer-product -> head-mix linear).

Reference computation (B=2, L=512, D=256, DH=32, H=16, K=2, P=128):
    x   = LayerNorm(node) * gamma + beta, masked        [B, L, D]
    left  = (x @ W_left + b_left)                       [B, L, DH] -> [B,L,H,K]
    right = (x @ W_right + b_right)/sqrt(DH)            [B, L, DH] -> [B,L,H,K]
    out[b,i,j,h] = sum_k left[b,i,h,k]*right[b,j,h,k]
    out[b,i,j,p] = sum_h out[b,i,j,h]*W_out[h,p] + b_out[p]   [B, L, L, P]

Mathematical restructuring (c = (h,k) combined channel, 0..31):
    out[b,i,j,p] = sum_c right[b,j,c] * (left[b,i,c] * W2[c,p]) + b_out[p]
with W2[c,p] = W_out[c//2, p].  M-packs M[c, (q,p)] = left[b,i_q,c]*W2[c,p]
for 4 i's are built in one broadcast multiply each; the pair matmul is
bf16 x bf16 -> fp32 PSUM:  lhsT=rightT[33, j-chunk 128] x rhs=M_pack[33, 512].
Row 32 of rightT is constant 1 and row 32 of the M-pack is b_out, which adds
the bias inside the same matmul.

PSUM is drained in [128, 1024] bank pairs with an f32->bf16 converting copy
(DVE/ACT alternating) into 1 MiB staging tiles, DMA'd per j-chunk.  The host
converts bf16 back to fp32 while assembling (bf16 + bf16-matmul rounding is
~6e-3 max-rel, inside the 2e-2 gate).

Sharding: the i axis of L is split across the 8 cores (sequence-parallel);
each core holds its [B, 64] slice of `left` plus the full `right` side and
writes a [B, 64, L, P] output shard.  No cross-device communication.

LayerNorm gamma/beta are folded into the projection weights on the host
(exact algebra): W_e = gamma[:,None]*W, with an extra K=1 accumulation row
carrying beta@W * mask.
"""

import os
import sys

sys.path.insert(0, "/opt/trn_rl_repo")

import numpy as np

import concourse.bass as bass
import concourse.mybir as mybir
import concourse.tile as tile
from concourse import bacc
from concourse.bass_utils import run_bass_kernel_spmd
from concourse.masks import make_identity

F32 = mybir.dt.float32
F32R = mybir.dt.float32r
BF16 = mybir.dt.bfloat16

B, L, D = 2, 512, 256
DH, H, PAIR = 32, 16, 128
NCORES = 8
LSH = L // NCORES          # 64 i's per core per batch
LN_EPS = 1e-5

# packed-constant column maps: cst_main [128, 267] + cst_rows [1, 1216]
COL_WL = (0, 32)           # [128, 32] x2: gamma*W_l rows 0-127 / 128-255
COL_WR = (64, 96)
COL_W2 = 128               # [33, 128] (cast to bf16 on chip)
COL_MCF = 256              # [128, 8]
COL_MCS = 264              # [128, 1]
COL_BL = 265               # [32, 1]
COL_BR = 266               # [32, 1]
NCONST = 267
ROW_WLR = 0                # [1, 32]  row 256 of w_left_e
ROW_WRR = 32               # [1, 32]
ROW_MRS = 64               # [1, 128]
ROW_MRF = (192, 704)       # [1, 512] x2
NROWS = 1216

_COMPILED = None  # (nc, input_names)


def _build_program():
    nc = bacc.Bacc("TRN2", target_bir_lowering=False, debug=False,
                   num_devices=NCORES)

    node_full = nc.dram_tensor("node_full", [B * L, D], F32,
                               kind="ExternalInput").ap()
    node_shard = nc.dram_tensor("node_shard", [B * LSH, D], F32,
                                kind="ExternalInput").ap()
    consts = nc.dram_tensor("consts", [128, NCONST], F32,
                            kind="ExternalInput").ap()
    const_rows = nc.dram_tensor("const_rows", [1, NROWS], F32,
                                kind="ExternalInput").ap()

    # Permuted output layout: [b, jc, q2, j, s, i16, p] (bf16) — each staging
    # buffer lands as one fully contiguous 1 MiB stream (8 KiB per partition
    # run).  sg = q2*2 + s; i_local = sg*16 + i16.  The host un-permutes +
    # upcasts while assembling the full output.
    out = nc.dram_tensor("out", [B, 4, 2, 128, 2, 16, PAIR], BF16,
                         kind="ExternalOutput").ap()

    with tile.TileContext(nc) as tc:
        with (
            tc.tile_pool(name="singles", bufs=1) as singles,
            tc.tile_pool(name="xpool", bufs=2) as xpool,
            tc.tile_pool(name="stats", bufs=4) as stats,
            tc.tile_pool(name="persist", bufs=1) as persist,
            tc.tile_pool(name="mp", bufs=32) as mp_pool,
            tc.tile_pool(name="stag", bufs=8) as stag_pool,
            tc.tile_pool(name="ps_big", bufs=4, space="PSUM") as ps_big,
        ):
            # ---------------- input loads (2 HWDGE rings, batched) ----------
            # One DMA per 4-tile half of node_full: a single completion
            # receipt instead of four ~2us ones.
            xs = xpool.tile([128, D], F32, tag="x", name="xs")
            nc.sync.dma_start(out=xs, in_=node_shard[:, :])
            cst = singles.tile([128, NCONST], F32, tag="cst")
            nc.scalar.dma_start(out=cst, in_=consts[:, :])
            crow = singles.tile([1, NROWS], F32, tag="crow")
            nc.scalar.dma_start(out=crow, in_=const_rows[:, :])
            xhalf = []
            for h, q in ((0, nc.sync), (1, nc.scalar)):
                xh = xpool.tile([128, 4 * D], F32, tag="xh", name=f"xh{h}")
                src_ap = node_full[h * 512:(h + 1) * 512, :].rearrange(
                    "(t j) d -> j t d", j=128)
                q.dma_start(out=xh.rearrange("j (t d) -> j t d", d=D),
                            in_=src_ap)
                xhalf.append(xh)
            xf_tiles = [xhalf[t // 4][:, (t % 4) * D:(t % 4 + 1) * D]
                        for t in range(8)]

            # ---------------- constants / views ----------------
            ident = singles.tile([128, 128], F32, tag="ident")
            make_identity(nc, ident)
            eps_t = singles.tile([128, 1], F32, tag="eps")
            nc.vector.memset(eps_t, LN_EPS)

            wl_sb = [cst[:, COL_WL[dc]:COL_WL[dc] + DH] for dc in range(2)]
            wr_sb = [cst[:, COL_WR[dc]:COL_WR[dc] + DH] for dc in range(2)]
            wl_row = crow[0:1, ROW_WLR:ROW_WLR + DH]
            wr_row = crow[0:1, ROW_WRR:ROW_WRR + DH]
            bl_sb = cst[0:DH, COL_BL:COL_BL + 1]
            br_sb = cst[0:DH, COL_BR:COL_BR + 1]
            mcf_sb = cst[:, COL_MCF:COL_MCF + 8]
            mcs_sb = cst[:, COL_MCS:COL_MCS + 1]
            mrs_sb = crow[0:1, ROW_MRS:ROW_MRS + B * LSH]
            mrf_sb = [crow[0:1, ROW_MRF[b]:ROW_MRF[b] + L] for b in range(B)]

            w2bf = singles.tile([DH + 1, PAIR], BF16, tag="w2bf")
            nc.scalar.copy(out=w2bf, in_=cst[0:DH + 1, COL_W2:COL_W2 + PAIR])

            # ---------------- LayerNorm helper ----------------
            def layernorm_masked(x_t, mask_col_ap):
                """x_t [128, D] in place -> (x - mu) * rsqrt(var+eps) * mask."""
                st = stats.tile([128, 6], F32, tag="st")
                nc.vector.bn_stats(out=st, in_=x_t)
                mv = stats.tile([128, 2], F32, tag="mv")
                nc.vector.bn_aggr(out=mv, in_=st)
                sd = stats.tile([128, 1], F32, tag="sd")
                nc.scalar.activation(out=sd, in_=mv[:, 1:2],
                                     func=mybir.ActivationFunctionType.Sqrt,
                                     bias=eps_t, scale=1.0)
                rs = stats.tile([128, 1], F32, tag="rs")
                nc.vector.reciprocal(out=rs, in_=sd)
                rsm = stats.tile([128, 1], F32, tag="rsm")
                nc.vector.tensor_mul(out=rsm, in0=rs, in1=mask_col_ap)
                nc.vector.tensor_scalar(out=x_t, in0=x_t,
                                        scalar1=mv[:, 0:1], scalar2=rsm,
                                        op0=mybir.AluOpType.subtract,
                                        op1=mybir.AluOpType.mult)

            # ---------------- shard path: leftT_all [33, B*LSH] bf16 --------
            layernorm_masked(xs, mcs_sb)

            xsT = [persist.tile([128, B * LSH], F32, tag=f"xsT{dc}",
                                name=f"xsT{dc}") for dc in range(2)]
            for dc in range(2):
                pt = ps_big.tile([128, 1024], F32, tag="big",
                                 name=f"tps{dc}")[:, 0:128]
                nc.tensor.transpose(pt, xs[:, dc * 128:(dc + 1) * 128], ident)
                nc.scalar.copy(out=xsT[dc], in_=pt)

            ps_l = ps_big.tile([128, 1024], F32, tag="big",
                               name="ps_l")[0:DH, 0:B * LSH]
            for dc in range(2):
                nc.tensor.matmul(ps_l, wl_sb[dc], xsT[dc],
                                 start=(dc == 0), stop=False)
            nc.tensor.matmul(ps_l, wl_row, mrs_sb, start=False, stop=True)
            leftT = persist.tile([DH + 1, B * LSH], BF16, tag="leftT")
            nc.scalar.activation(out=leftT[0:DH, :], in_=ps_l,
                                 func=mybir.ActivationFunctionType.Identity,
                                 bias=bl_sb, scale=1.0)
            nc.vector.memset(leftT[DH:DH + 1, :], 1.0)

            # ---------------- full path (per batch): rightT[b] [33, L] bf16 -
            rightT = [persist.tile([DH + 1, L], BF16, tag=f"rt{b}",
                                   name=f"rt{b}") for b in range(B)]
            xT = [[persist.tile([128, L], F32, tag=f"xT{b}_{dc}",
                                name=f"xT{b}_{dc}") for dc in range(2)]
                  for b in range(B)]

            def build_right(b):
                for lc in range(4):
                    t = b * 4 + lc
                    xf = xf_tiles[t]
                    layernorm_masked(xf, mcf_sb[:, t:t + 1])
                    for dc in range(2):
                        pt = ps_big.tile([128, 1024], F32, tag="big",
                                         name=f"tp{b}_{lc}_{dc}")[:, 0:128]
                        nc.tensor.transpose(pt, xf[:, dc * 128:(dc + 1) * 128],
                                            ident)
                        nc.scalar.copy(
                            out=xT[b][dc][:, lc * 128:(lc + 1) * 128],
                            in_=pt)
                    # project this j-chunk as soon as its columns exist
                    jc = lc
                    jsl = slice(jc * 128, (jc + 1) * 128)
                    prj = ps_big.tile([128, 1024], F32, tag="big",
                                      name=f"prj{b}_{jc}")[0:DH, 0:128]
                    for dc in range(2):
                        nc.tensor.matmul(prj, wr_sb[dc],
                                         xT[b][dc][:, jsl],
                                         start=(dc == 0), stop=False)
                    nc.tensor.matmul(prj, wr_row, mrf_sb[b][:, jsl],
                                     start=False, stop=True)
                    nc.scalar.activation(out=rightT[b][0:DH, jsl],
                                         in_=prj,
                                         func=mybir.ActivationFunctionType.Identity,
                                         bias=br_sb, scale=1.0)
                nc.vector.memset(rightT[b][DH:DH + 1, :], 1.0)

            # ---------------- main pair loop ----------------
            def build_mp4(b, sg, il):
                """[33, 512] bf16 pack for 4 i's:
                mp[c, q*128 + p] = left[b, i(sg,il,q), c] * w2[c, p]."""
                mp = mp_pool.tile([DH + 1, 512], BF16, tag="mp",
                                  name=f"mp{b}_{sg}_{il}")
                col = b * LSH + (sg * 4 + il) * 4
                lsrc = leftT[:, col:col + 4].unsqueeze(-1).to_broadcast(
                    [DH + 1, 4, PAIR])
                wsrc = w2bf[:, :].unsqueeze(1).to_broadcast([DH + 1, 4, PAIR])
                dst = mp[:, :].rearrange("c (q p) -> c q p", p=PAIR)
                nc.gpsimd.tensor_mul(out=dst, in0=wsrc, in1=lsrc)
                return mp

            # all 32 M-packs depend only on leftT; hoist them in round order
            # on the (otherwise idle) GpSimd queue so no round ever waits.
            all_mps = {}

            def build_all_mps():
                for b in range(B):
                    for q2 in range(2):
                        for s in range(2):
                            for il in range(4):
                                all_mps[(b, q2, s, il)] = build_mp4(
                                    b, q2 * 2 + s, il)

            def pair_rounds(b):
                for q2 in range(2):
                    mps = [[all_mps[(b, q2, s, il)] for il in range(4)]
                           for s in range(2)]
                    for jc in range(4):
                        lhsT = rightT[b][:, jc * 128:(jc + 1) * 128]
                        stg = stag_pool.tile([128, 2 * 16 * PAIR], BF16,
                                             tag="stag")
                        for s in range(2):
                            for ilp in range(2):
                                pb = ps_big.tile([128, 1024], F32, tag="big")
                                nc.tensor.matmul(pb[:, 0:512], lhsT,
                                                 mps[s][2 * ilp],
                                                 start=True, stop=True)
                                nc.tensor.matmul(pb[:, 512:1024], lhsT,
                                                 mps[s][2 * ilp + 1],
                                                 start=True, stop=True)
                                dst = stg[:, (s * 16 + ilp * 8) * PAIR:
                                          (s * 16 + ilp * 8 + 8) * PAIR]
                                if (s + ilp) % 2 == 0:
                                    nc.vector.tensor_copy(out=dst, in_=pb)
                                else:
                                    nc.scalar.copy(out=dst, in_=pb)
                        dst_ap = out[b, jc, q2, :, :, :, :]
                        src_ap = stg[:, :].rearrange("j (s i p) -> j s i p",
                                                     s=2, p=PAIR)
                        deng = nc.sync if jc % 2 == 0 else nc.scalar
                        deng.dma_start(out=dst_ap, in_=src_ap)

            build_right(0)
            build_right(1)
            build_all_mps()
            pair_rounds(0)
            pair_rounds(1)

    nc.compile()
    names = ["node_full", "node_shard", "consts"]
    return nc, names


def _prepare_in_maps(node, mask, ln_gamma, ln_beta, W_left, b_left, W_right,
                     b_right, W_out, b_out):
    f = np.float32
    node = np.ascontiguousarray(np.asarray(node, dtype=f))        # [B, L, D]
    mask_f = np.asarray(mask).astype(f)                           # [B, L]
    gamma = np.asarray(ln_gamma, dtype=f)
    beta = np.asarray(ln_beta, dtype=f)
    W_l = np.asarray(W_left, dtype=f)
    W_r = np.asarray(W_right, dtype=f)
    b_l = np.asarray(b_left, dtype=f)
    b_r = np.asarray(b_right, dtype=f)
    W_o = np.asarray(W_out, dtype=f)
    b_o = np.asarray(b_out, dtype=f)

    s = 1.0 / np.sqrt(np.float32(DH))
    w_left_e = np.concatenate([gamma[:, None] * W_l, (beta @ W_l)[None, :]], 0)
    w_right_e = np.concatenate([gamma[:, None] * W_r, (beta @ W_r)[None, :]],
                               0) * s
    w2 = np.concatenate([np.repeat(W_o, 2, axis=0), b_o[None, :]], 0)

    common_cst = np.zeros((128, NCONST), f)
    for dc in range(2):
        common_cst[:, COL_WL[dc]:COL_WL[dc] + DH] = \
            w_left_e[dc * 128:(dc + 1) * 128]
        common_cst[:, COL_WR[dc]:COL_WR[dc] + DH] = \
            w_right_e[dc * 128:(dc + 1) * 128]
    common_cst[0:DH + 1, COL_W2:COL_W2 + PAIR] = w2
    common_cst[:, COL_MCF:COL_MCF + 8] = mask_f.reshape(-1, 128).T
    common_cst[0:DH, COL_BL] = b_l
    common_cst[0:DH, COL_BR] = b_r * s

    common_rows = np.zeros((1, NROWS), f)
    common_rows[0, ROW_WLR:ROW_WLR + DH] = w_left_e[D]
    common_rows[0, ROW_WRR:ROW_WRR + DH] = w_right_e[D]
    for b in range(B):
        common_rows[0, ROW_MRF[b]:ROW_MRF[b] + L] = mask_f[b]

    node_flat = node.reshape(B * L, D)

    in_maps = []
    for c in range(NCORES):
        sl = slice(c * LSH, (c + 1) * LSH)
        shard = np.ascontiguousarray(node[:, sl, :].reshape(B * LSH, D))
        msk = mask_f[:, sl]                                       # [B, LSH]
        cstc = common_cst.copy()
        cstc[:, COL_MCS] = msk.reshape(-1)
        rowc = common_rows.copy()
        rowc[0, ROW_MRS:ROW_MRS + B * LSH] = msk.reshape(-1)
        in_maps.append({
            "node_full": node_flat,
            "node_shard": shard,
            "consts": cstc,
            "const_rows": rowc,
        })
    return in_maps


def kernel(**inputs):
    global _COMPILED
    if _COMPILED is None:
        _COMPILED = _build_program()
    nc, names = _COMPILED
    in_maps = _prepare_in_maps(**inputs)
    res = run_bass_kernel_spmd(nc, in_maps, core_ids=list(range(NCORES)))
    full = np.empty((B, L, L, PAIR), np.float32)
    for c in range(NCORES):
        dev = res.results[c]["out"]   # [b, jc, q2, j, s, i16, p] bf16
        full[:, c * LSH:(c + 1) * LSH] = (
            dev.transpose(0, 2, 4, 5, 1, 3, 6)
               .reshape(B, LSH, L, PAIR).astype(np.float32))
    return full


if __name__ == "__main__":
    # self-test with NON-trivial gamma/beta/mask against a numpy reference
    rng = np.random.default_rng(1)
    mask = np.ones((B, L), dtype=bool)
    mask[0, 500:] = False        # exercise the mask path
    mask[1, :3] = False
    inputs = {
        "node": rng.standard_normal((B, L, D)).astype(np.float32),
        "mask": mask,
        "ln_gamma": (1.0 + 0.1 * rng.standard_normal(D)).astype(np.float32),
        "ln_beta": (0.1 * rng.standard_normal(D)).astype(np.float32),
        "W_left": (rng.standard_normal((D, DH)) / np.sqrt(D)).astype(np.float32),
        "b_left": (0.1 * rng.standard_normal(DH)).astype(np.float32),
        "W_right": (rng.standard_normal((D, DH)) / np.sqrt(D)).astype(np.float32),
        "b_right": (0.1 * rng.standard_normal(DH)).astype(np.float32),
        "W_out": (rng.standard_normal((H, PAIR)) / np.sqrt(H)).astype(np.float32),
        "b_out": (0.1 * rng.standard_normal(PAIR)).astype(np.float32),
    }

    def np_reference(node, mask, ln_gamma, ln_beta, W_left, b_left, W_right,
                     b_right, W_out, b_out):
        node = node.astype(np.float64)
        mu = node.mean(-1, keepdims=True)
        var = ((node - mu) ** 2).mean(-1, keepdims=True)
        x = (node - mu) / np.sqrt(var + LN_EPS) * ln_gamma + ln_beta
        x = x * mask[..., None]
        left = (x @ W_left + b_left).reshape(B, L, H, -1)
        right = ((x @ W_right + b_right) / np.sqrt(DH)).reshape(B, L, H, -1)
        o = np.einsum("bihk,bjhk->bijh", left, right)
        return np.einsum("bijh,hp->bijp", o, W_out) + b_out

    got = kernel(**inputs)
    exp = np_reference(**inputs)
    rel = np.abs(got - exp).max() / np.abs(exp).max()
    print("general-path rel err:", rel)
    assert rel < 1.5e-2, rel
    print("OK", got.shape, got.dtype)


# revision 19
# speedup vs baseline: 1.1545x; 1.1321x over previous
"""Trainium2 Bass kernel for nn_Node2Pair_bias (LayerNorm -> dual projection ->
pair outer-product -> head-mix linear).

Reference computation (B=2, L=512, D=256, DH=32, H=16, K=2, P=128):
    x   = LayerNorm(node) * gamma + beta, masked        [B, L, D]
    left  = (x @ W_left + b_left)                       [B, L, DH] -> [B,L,H,K]
    right = (x @ W_right + b_right)/sqrt(DH)            [B, L, DH] -> [B,L,H,K]
    out[b,i,j,h] = sum_k left[b,i,h,k]*right[b,j,h,k]
    out[b,i,j,p] = sum_h out[b,i,j,h]*W_out[h,p] + b_out[p]   [B, L, L, P]

Mathematical restructuring (c = (h,k) combined channel, 0..31):
    out[b,i,j,p] = sum_c right[b,j,c] * (left[b,i,c] * W2[c,p]) + b_out[p]
with W2[c,p] = W_out[c//2, p].  M-packs M[c, (q,p)] = left[b,i_q,c]*W2[c,p]
for 4 i's are built in one broadcast multiply each; the pair matmul is
bf16 x bf16 -> fp32 PSUM:  lhsT=rightT[33, j-chunk 128] x rhs=M_pack[33, 512].
Row 32 of rightT is constant 1 and row 32 of the M-pack is b_out, which adds
the bias inside the same matmul.

PSUM is drained in [128, 1024] bank pairs with an f32->bf16 converting copy
(DVE/ACT alternating) into 1 MiB staging tiles, DMA'd per j-chunk.  The host
converts bf16 back to fp32 while assembling (bf16 + bf16-matmul rounding is
~6e-3 max-rel, inside the 2e-2 gate).

Sharding: the i axis of L is split across the 8 cores (sequence-parallel);
each core holds its [B, 64] slice of `left` plus the full `right` side and
writes a [B, 64, L, P] output shard.  No cross-device communication.

LayerNorm gamma/beta are folded into the projection weights on the host
(exact algebra): W_e = gamma[:,None]*W, with an extra K=1 accumulation row
carrying beta@W * mask.
"""

import os
import sys

sys.path.insert(0, "/opt/trn_rl_repo")

import numpy as np

import concourse.bass as bass
import concourse.mybir as mybir
import concourse.tile as tile
from concourse import bacc
from concourse.bass_utils import run_bass_kernel_spmd
from concourse.masks import make_identity

F32 = mybir.dt.float32
F32R = mybir.dt.float32r
BF16 = mybir.dt.bfloat16

B, L, D = 2, 512, 256
DH, H, PAIR = 32, 16, 128
NCORES = 8
LSH = L // NCORES          # 64 i's per core per batch
LN_EPS = 1e-5

# packed-constant column maps: cst_main [128, 267] + cst_rows [1, 1216]
COL_WL = (0, 32)           # [128, 32] x2: gamma*W_l rows 0-127 / 128-255
COL_WR = (64, 96)
COL_W2 = 128               # [33, 128] (cast to bf16 on chip)
COL_MCF = 256              # [128, 8]
COL_MCS = 264              # [128, 1]
COL_BL = 265               # [32, 1]
COL_BR = 266               # [32, 1]
NCONST = 267
ROW_WLR = 0                # [1, 32]  row 256 of w_left_e
ROW_WRR = 32               # [1, 32]
ROW_MRS = 64               # [1, 128]
ROW_MRF = (192, 704)       # [1, 512] x2
NROWS = 1216

_COMPILED = None  # (nc, input_names)


def _build_program():
    nc = bacc.Bacc("TRN2", target_bir_lowering=False, debug=False,
                   num_devices=NCORES)

    node_full = nc.dram_tensor("node_full", [B * L, D], F32,
                               kind="ExternalInput").ap()
    node_shard = nc.dram_tensor("node_shard", [B * LSH, D], F32,
                                kind="ExternalInput").ap()
    consts = nc.dram_tensor("consts", [128, NCONST], F32,
                            kind="ExternalInput").ap()
    const_rows = nc.dram_tensor("const_rows", [1, NROWS], F32,
                                kind="ExternalInput").ap()

    # Permuted output layout: [b, jc, q2, j, s, i16, p] (bf16) — each staging
    # buffer lands as one fully contiguous 1 MiB stream (8 KiB per partition
    # run).  sg = q2*2 + s; i_local = sg*16 + i16.  The host un-permutes +
    # upcasts while assembling the full output.
    out = nc.dram_tensor("out", [B, 4, 2, 128, 2, 16, PAIR], BF16,
                         kind="ExternalOutput").ap()

    with tile.TileContext(nc) as tc:
        with (
            tc.tile_pool(name="singles", bufs=1) as singles,
            tc.tile_pool(name="xpool", bufs=2) as xpool,
            tc.tile_pool(name="stats", bufs=4) as stats,
            tc.tile_pool(name="persist", bufs=1) as persist,
            tc.tile_pool(name="mp", bufs=32) as mp_pool,
            tc.tile_pool(name="stag", bufs=8) as stag_pool,
            tc.tile_pool(name="ps_big", bufs=8, space="PSUM") as ps_big,
        ):
            # ---------------- input loads (2 HWDGE rings, batched) ----------
            # One DMA per 4-tile half of node_full: a single completion
            # receipt instead of four ~2us ones.
            xs = xpool.tile([128, D], F32, tag="x", name="xs")
            nc.sync.dma_start(out=xs, in_=node_shard[:, :])
            cst = singles.tile([128, NCONST], F32, tag="cst")
            nc.scalar.dma_start(out=cst, in_=consts[:, :])
            crow = singles.tile([1, NROWS], F32, tag="crow")
            nc.scalar.dma_start(out=crow, in_=const_rows[:, :])
            xhalf = []
            for h, q in ((0, nc.sync), (1, nc.scalar)):
                xh = xpool.tile([128, 4 * D], F32, tag="xh", name=f"xh{h}")
                src_ap = node_full[h * 512:(h + 1) * 512, :].rearrange(
                    "(t j) d -> j t d", j=128)
                q.dma_start(out=xh.rearrange("j (t d) -> j t d", d=D),
                            in_=src_ap)
                xhalf.append(xh)
            xf_tiles = [xhalf[t // 4][:, (t % 4) * D:(t % 4 + 1) * D]
                        for t in range(8)]

            # ---------------- constants / views ----------------
            ident = singles.tile([128, 128], F32, tag="ident")
            make_identity(nc, ident)
            eps_t = singles.tile([128, 1], F32, tag="eps")
            nc.vector.memset(eps_t, LN_EPS)

            wl_sb = [cst[:, COL_WL[dc]:COL_WL[dc] + DH] for dc in range(2)]
            wr_sb = [cst[:, COL_WR[dc]:COL_WR[dc] + DH] for dc in range(2)]
            wl_row = crow[0:1, ROW_WLR:ROW_WLR + DH]
            wr_row = crow[0:1, ROW_WRR:ROW_WRR + DH]
            bl_sb = cst[0:DH, COL_BL:COL_BL + 1]
            br_sb = cst[0:DH, COL_BR:COL_BR + 1]
            mcf_sb = cst[:, COL_MCF:COL_MCF + 8]
            mcs_sb = cst[:, COL_MCS:COL_MCS + 1]
            mrs_sb = crow[0:1, ROW_MRS:ROW_MRS + B * LSH]
            mrf_sb = [crow[0:1, ROW_MRF[b]:ROW_MRF[b] + L] for b in range(B)]

            w2bf = singles.tile([DH + 1, PAIR], BF16, tag="w2bf")
            nc.scalar.copy(out=w2bf, in_=cst[0:DH + 1, COL_W2:COL_W2 + PAIR])

            # ---------------- LayerNorm helper ----------------
            def layernorm_masked(x_t, mask_col_ap):
                """x_t [128, D] in place -> (x - mu) * rsqrt(var+eps) * mask."""
                st = stats.tile([128, 6], F32, tag="st")
                nc.vector.bn_stats(out=st, in_=x_t)
                mv = stats.tile([128, 2], F32, tag="mv")
                nc.vector.bn_aggr(out=mv, in_=st)
                sd = stats.tile([128, 1], F32, tag="sd")
                nc.scalar.activation(out=sd, in_=mv[:, 1:2],
                                     func=mybir.ActivationFunctionType.Sqrt,
                                     bias=eps_t, scale=1.0)
                rs = stats.tile([128, 1], F32, tag="rs")
                nc.vector.reciprocal(out=rs, in_=sd)
                rsm = stats.tile([128, 1], F32, tag="rsm")
                nc.vector.tensor_mul(out=rsm, in0=rs, in1=mask_col_ap)
                nc.vector.tensor_scalar(out=x_t, in0=x_t,
                                        scalar1=mv[:, 0:1], scalar2=rsm,
                                        op0=mybir.AluOpType.subtract,
                                        op1=mybir.AluOpType.mult)

            # ---------------- shard path: leftT_all [33, B*LSH] bf16 --------
            layernorm_masked(xs, mcs_sb)

            xsT = [persist.tile([128, B * LSH], F32, tag=f"xsT{dc}",
                                name=f"xsT{dc}") for dc in range(2)]
            for dc in range(2):
                pt = ps_big.tile([128, 512], F32, tag="big",
                                 name=f"tps{dc}")[:, 0:128]
                nc.tensor.transpose(pt, xs[:, dc * 128:(dc + 1) * 128], ident)
                nc.scalar.copy(out=xsT[dc], in_=pt)

            ps_l = ps_big.tile([128, 512], F32, tag="big",
                               name="ps_l")[0:DH, 0:B * LSH]
            for dc in range(2):
                nc.tensor.matmul(ps_l, wl_sb[dc], xsT[dc],
                                 start=(dc == 0), stop=False)
            nc.tensor.matmul(ps_l, wl_row, mrs_sb, start=False, stop=True)
            leftT = persist.tile([DH + 1, B * LSH], BF16, tag="leftT")
            nc.scalar.activation(out=leftT[0:DH, :], in_=ps_l,
                                 func=mybir.ActivationFunctionType.Identity,
                                 bias=bl_sb, scale=1.0)
            nc.vector.memset(leftT[DH:DH + 1, :], 1.0)

            # ---------------- full path (per batch): rightT[b] [33, L] bf16 -
            rightT = [persist.tile([DH + 1, L], BF16, tag=f"rt{b}",
                                   name=f"rt{b}") for b in range(B)]
            xT = [[persist.tile([128, L], F32, tag=f"xT{b}_{dc}",
                                name=f"xT{b}_{dc}") for dc in range(2)]
                  for b in range(B)]

            def build_right(b):
                for lc in range(4):
                    t = b * 4 + lc
                    xf = xf_tiles[t]
                    layernorm_masked(xf, mcf_sb[:, t:t + 1])
                    for dc in range(2):
                        pt = ps_big.tile([128, 512], F32, tag="big",
                                         name=f"tp{b}_{lc}_{dc}")[:, 0:128]
                        nc.tensor.transpose(pt, xf[:, dc * 128:(dc + 1) * 128],
                                            ident)
                        nc.scalar.copy(
                            out=xT[b][dc][:, lc * 128:(lc + 1) * 128],
                            in_=pt)
                    # project this j-chunk as soon as its columns exist
                    jc = lc
                    jsl = slice(jc * 128, (jc + 1) * 128)
                    prj = ps_big.tile([128, 512], F32, tag="big",
                                      name=f"prj{b}_{jc}")[0:DH, 0:128]
                    for dc in range(2):
                        nc.tensor.matmul(prj, wr_sb[dc],
                                         xT[b][dc][:, jsl],
                                         start=(dc == 0), stop=False)
                    nc.tensor.matmul(prj, wr_row, mrf_sb[b][:, jsl],
                                     start=False, stop=True)
                    nc.scalar.activation(out=rightT[b][0:DH, jsl],
                                         in_=prj,
                                         func=mybir.ActivationFunctionType.Identity,
                                         bias=br_sb, scale=1.0)
                nc.vector.memset(rightT[b][DH:DH + 1, :], 1.0)

            # ---------------- main pair loop ----------------
            def build_mp4(b, sg, il):
                """[33, 512] bf16 pack for 4 i's:
                mp[c, q*128 + p] = left[b, i(sg,il,q), c] * w2[c, p]."""
                mp = mp_pool.tile([DH + 1, 512], BF16, tag="mp",
                                  name=f"mp{b}_{sg}_{il}")
                col = b * LSH + (sg * 4 + il) * 4
                lsrc = leftT[:, col:col + 4].unsqueeze(-1).to_broadcast(
                    [DH + 1, 4, PAIR])
                wsrc = w2bf[:, :].unsqueeze(1).to_broadcast([DH + 1, 4, PAIR])
                dst = mp[:, :].rearrange("c (q p) -> c q p", p=PAIR)
                nc.gpsimd.tensor_mul(out=dst, in0=wsrc, in1=lsrc)
                return mp

            # all 32 M-packs depend only on leftT; hoist them in round order
            # on the (otherwise idle) GpSimd queue so no round ever waits.
            all_mps = {}

            def build_all_mps():
                for b in range(B):
                    for q2 in range(2):
                        for s in range(2):
                            for il in range(4):
                                all_mps[(b, q2, s, il)] = build_mp4(
                                    b, q2 * 2 + s, il)

            def pair_rounds(b):
                for q2 in range(2):
                    mps = [[all_mps[(b, q2, s, il)] for il in range(4)]
                           for s in range(2)]
                    for jc in range(4):
                        lhsT = rightT[b][:, jc * 128:(jc + 1) * 128]
                        stg = stag_pool.tile([128, 2 * 16 * PAIR], BF16,
                                             tag="stag")
                        for s in range(2):
                            for il in range(4):
                                pb = ps_big.tile([128, 512], F32, tag="big")
                                nc.tensor.matmul(pb, lhsT, mps[s][il],
                                                 start=True, stop=True)
                                dst = stg[:, (s * 16 + il * 4) * PAIR:
                                          (s * 16 + il * 4 + 4) * PAIR]
                                if (s * 4 + il) % 2 == 0:
                                    nc.vector.tensor_copy(out=dst, in_=pb)
                                else:
                                    nc.scalar.copy(out=dst, in_=pb)
                        dst_ap = out[b, jc, q2, :, :, :, :]
                        src_ap = stg[:, :].rearrange("j (s i p) -> j s i p",
                                                     s=2, p=PAIR)
                        nc.sync.dma_start(out=dst_ap, in_=src_ap)

            build_right(0)
            build_right(1)
            build_all_mps()
            pair_rounds(0)
            pair_rounds(1)

    nc.compile()
    names = ["node_full", "node_shard", "consts"]
    return nc, names


def _prepare_in_maps(node, mask, ln_gamma, ln_beta, W_left, b_left, W_right,
                     b_right, W_out, b_out):
    f = np.float32
    node = np.ascontiguousarray(np.asarray(node, dtype=f))        # [B, L, D]
    mask_f = np.asarray(mask).astype(f)                           # [B, L]
    gamma = np.asarray(ln_gamma, dtype=f)
    beta = np.asarray(ln_beta, dtype=f)
    W_l = np.asarray(W_left, dtype=f)
    W_r = np.asarray(W_right, dtype=f)
    b_l = np.asarray(b_left, dtype=f)
    b_r = np.asarray(b_right, dtype=f)
    W_o = np.asarray(W_out, dtype=f)
    b_o = np.asarray(b_out, dtype=f)

    s = 1.0 / np.sqrt(np.float32(DH))
    w_left_e = np.concatenate([gamma[:, None] * W_l, (beta @ W_l)[None, :]], 0)
    w_right_e = np.concatenate([gamma[:, None] * W_r, (beta @ W_r)[None, :]],
                               0) * s
    w2 = np.concatenate([np.repeat(W_o, 2, axis=0), b_o[None, :]], 0)

    common_cst = np.zeros((128, NCONST), f)
    for dc in range(2):
        common_cst[:, COL_WL[dc]:COL_WL[dc] + DH] = \
            w_left_e[dc * 128:(dc + 1) * 128]
        common_cst[:, COL_WR[dc]:COL_WR[dc] + DH] = \
            w_right_e[dc * 128:(dc + 1) * 128]
    common_cst[0:DH + 1, COL_W2:COL_W2 + PAIR] = w2
    common_cst[:, COL_MCF:COL_MCF + 8] = mask_f.reshape(-1, 128).T
    common_cst[0:DH, COL_BL] = b_l
    common_cst[0:DH, COL_BR] = b_r * s

    common_rows = np.zeros((1, NROWS), f)
    common_rows[0, ROW_WLR:ROW_WLR + DH] = w_left_e[D]
    common_rows[0, ROW_WRR:ROW_WRR + DH] = w_right_e[D]
    for b in range(B):
        common_rows[0, ROW_MRF[b]:ROW_MRF[b] + L] = mask_f[b]

    node_flat = node.reshape(B * L, D)

    in_maps = []
    for c in range(NCORES):
        sl = slice(c * LSH, (c + 1) * LSH)
        shard = np.ascontiguousarray(node[:, sl, :].reshape(B * LSH, D))
        msk = mask_f[:, sl]                                       # [B, LSH]
        cstc = common_cst.copy()
        cstc[:, COL_MCS] = msk.reshape(-1)
        rowc = common_rows.copy()
        rowc[0, ROW_MRS:ROW_MRS + B * LSH] = msk.reshape(-1)
        in_maps.append({
            "node_full": node_flat,
            "node_shard": shard,
            "consts": cstc,
            "const_rows": rowc,
        })
    return in_maps


def kernel(**inputs):
    global _COMPILED
    if _COMPILED is None:
        _COMPILED = _build_program()
    nc, names = _COMPILED
    in_maps = _prepare_in_maps(**inputs)
    res = run_bass_kernel_spmd(nc, in_maps, core_ids=list(range(NCORES)))
    full = np.empty((B, L, L, PAIR), np.float32)
    for c in range(NCORES):
        dev = res.results[c]["out"]   # [b, jc, q2, j, s, i16, p] bf16
        full[:, c * LSH:(c + 1) * LSH] = (
            dev.transpose(0, 2, 4, 5, 1, 3, 6)
               .reshape(B, LSH, L, PAIR).astype(np.float32))
    return full


if __name__ == "__main__":
    # self-test with NON-trivial gamma/beta/mask against a numpy reference
    rng = np.random.default_rng(1)
    mask = np.ones((B, L), dtype=bool)
    mask[0, 500:] = False        # exercise the mask path
    mask[1, :3] = False
    inputs = {
        "node": rng.standard_normal((B, L, D)).astype(np.float32),
        "mask": mask,
        "ln_gamma": (1.0 + 0.1 * rng.standard_normal(D)).astype(np.float32),
        "ln_beta": (0.1 * rng.standard_normal(D)).astype(np.float32),
        "W_left": (rng.standard_normal((D, DH)) / np.sqrt(D)).astype(np.float32),
        "b_left": (0.1 * rng.standard_normal(DH)).astype(np.float32),
        "W_right": (rng.standard_normal((D, DH)) / np.sqrt(D)).astype(np.float32),
        "b_right": (0.1 * rng.standard_normal(DH)).astype(np.float32),
        "W_out": (rng.standard_normal((H, PAIR)) / np.sqrt(H)).astype(np.float32),
        "b_out": (0.1 * rng.standard_normal(PAIR)).astype(np.float32),
    }

    def np_reference(node, mask, ln_gamma, ln_beta, W_left, b_left, W_right,
                     b_right, W_out, b_out):
        node = node.astype(np.float64)
        mu = node.mean(-1, keepdims=True)
        var = ((node - mu) ** 2).mean(-1, keepdims=True)
        x = (node - mu) / np.sqrt(var + LN_EPS) * ln_gamma + ln_beta
        x = x * mask[..., None]
        left = (x @ W_left + b_left).reshape(B, L, H, -1)
        right = ((x @ W_right + b_right) / np.sqrt(DH)).reshape(B, L, H, -1)
        o = np.einsum("bihk,bjhk->bijh", left, right)
        return np.einsum("bijh,hp->bijp", o, W_out) + b_out

    got = kernel(**inputs)
    exp = np_reference(**inputs)
    rel = np.abs(got - exp).max() / np.abs(exp).max()
    print("general-path rel err:", rel)
    assert rel < 1.5e-2, rel
    print("OK", got.shape, got.dtype)


# revision 20
# speedup vs baseline: 1.2363x; 1.0708x over previous
"""Trainium2 Bass kernel for nn_Node2Pair_bias (LayerNorm -> dual projection ->
pair outer-product -> head-mix linear).

Reference computation (B=2, L=512, D=256, DH=32, H=16, K=2, P=128):
    x   = LayerNorm(node) * gamma + beta, masked        [B, L, D]
    left  = (x @ W_left + b_left)                       [B, L, DH] -> [B,L,H,K]
    right = ((x @ W_right + b_right)/sqrt(DH))          [B, L, DH] -> [B,L,H,K]
    out[b,i,j,h] = sum_k left[b,i,h,k]*right[b,j,h,k]
    out[b,i,j,p] = sum_h out[b,i,j,h]*W_out[h,p] + b_out[p]   [B, L, L, P]

Mathematical restructuring (c = (h,k) combined channel, 0..31):
    out[b,i,j,p] = sum_c right[b,j,c] * (left[b,i,c] * W2[c,p]) + b_out[p]
with W2[c,p] = W_out[c//2, p].  M-packs M[c, (q,p)] = left[b,i_q,c]*W2[c,p]
for 4 i's are built on GpSimd with one broadcast multiply each; the pair
matmul is bf16 x bf16 -> fp32 PSUM:  lhsT=rightT[33, j-chunk 128] x
rhs=M_pack[33, 512].  Row 32 of rightT is constant 1 and row 32 of the
M-pack is b_out, which adds the bias inside the same matmul.

The PE on this part streams at ~1.2 GHz (427ns per 512-col matmul,
measured), so PE column count is the kernel's hard floor; everything else
is arranged to keep the PE streaming continuously:
  - all x transposes are done by the DMA xbar (bf16 dma_start transpose)
    instead of the PE,
  - projections run in bf16 (1 cycle/row instead of 4),
  - PSUM is a single 8 x 1-bank pool so the matmul/drain ring never
    couples (one matmul per bank, 8 in flight),
  - drains [128,512] f32->bf16 alternate DVE(56)/ACT(72) by measured rate,
  - output DMA triggers stay off the drain queues: sync ring for b=0,
    gpsimd (SWDGE) for b=1 (queued after all M-packs).

Weights/masks ship as bf16 pairs packed in fp32 DRAM words, viewed on-chip
via AP.bitcast.  The host converts the bf16 output back to fp32 while
assembling.

Sharding: the i axis of L is split across the 8 cores (sequence-parallel);
each core holds its [B, 64] slice of `left` plus the full `right` side and
writes a [B, 64, L, P] output shard.  No cross-device communication.

LayerNorm gamma/beta are folded into the projection weights on the host
(exact algebra): W_e = gamma[:,None]*W, with an extra K=1 accumulation row
carrying beta@W * mask.
"""

import os
import sys

sys.path.insert(0, "/opt/trn_rl_repo")

import numpy as np

import concourse.bass as bass
import concourse.mybir as mybir
import concourse.tile as tile
from concourse import bacc
from concourse.bass_utils import run_bass_kernel_spmd

F32 = mybir.dt.float32
BF16 = mybir.dt.bfloat16

B, L, D = 2, 512, 256
DH, H, PAIR = 32, 16, 128
NCORES = 8
LSH = L // NCORES          # 64 i's per core per batch
LN_EPS = 1e-5

# packed-constant column maps.  bf16 payloads are packed pairwise into f32
# words (host .view(np.float32)); offsets below are in f32 columns.
CW_WL = (0, 16)            # [128, 32]bf16 x2: gamma*W_l rows 0-127 / 128-255
CW_WR = (32, 48)
CW_W2 = 64                 # [33, 128]bf16 (W_out rows repeated x2 + b_out)
CW_MCF = 128               # [128, 8] f32 LN column masks (full seq)
CW_MCS = 136               # [128, 1] f32 (shard)
CW_BL = 137                # [32, 1] f32
CW_BR = 138                # [32, 1] f32
NCONST = 139
RW_WLR = 0                 # [1, 32]bf16  row 256 of w_left_e
RW_WRR = 16
RW_MRS = 32                # [1, 128]bf16 shard mask row
RW_MRF = (96, 352)         # [1, 512]bf16 full mask rows x2
NROWS = 608

_COMPILED = None  # (nc, input_names)


def _build_program():
    nc = bacc.Bacc("TRN2", target_bir_lowering=False, debug=False,
                   num_devices=NCORES)

    node_full = nc.dram_tensor("node_full", [B * L, D], F32,
                               kind="ExternalInput").ap()
    node_shard = nc.dram_tensor("node_shard", [B * LSH, D], F32,
                                kind="ExternalInput").ap()
    consts = nc.dram_tensor("consts", [128, NCONST], F32,
                            kind="ExternalInput").ap()
    const_rows = nc.dram_tensor("const_rows", [1, NROWS], F32,
                                kind="ExternalInput").ap()

    # Permuted output layout: [b, jc, q2, j, s, i16, p] (bf16) — each staging
    # buffer lands as one fully contiguous 1 MiB stream (8 KiB per partition
    # run).  sg = q2*2 + s; i_local = sg*16 + i16.  The host un-permutes +
    # upcasts while assembling the full output.
    out = nc.dram_tensor("out", [B, 4, 2, 128, 2, 16, PAIR], BF16,
                         kind="ExternalOutput").ap()

    with tile.TileContext(nc) as tc:
        with (
            tc.tile_pool(name="singles", bufs=1) as singles,
            tc.tile_pool(name="xpool", bufs=2) as xpool,
            tc.tile_pool(name="xbpool", bufs=4) as xbpool,
            tc.tile_pool(name="stats", bufs=4) as stats,
            tc.tile_pool(name="persist", bufs=1) as persist,
            tc.tile_pool(name="mp", bufs=32) as mp_pool,
            tc.tile_pool(name="stag", bufs=8) as stag_pool,
            tc.tile_pool(name="ps_big", bufs=8, space="PSUM") as ps_big,
        ):
            # ---------------- input loads (2 HWDGE rings, batched) ----------
            xs = xpool.tile([128, D], F32, tag="x", name="xs")
            nc.sync.dma_start(out=xs, in_=node_shard[:, :])
            cst = singles.tile([128, NCONST], F32, tag="cst")
            nc.scalar.dma_start(out=cst, in_=consts[:, :])
            crow = singles.tile([1, NROWS], F32, tag="crow")
            nc.scalar.dma_start(out=crow, in_=const_rows[:, :])
            xhalf = []
            for h, q in ((0, nc.sync), (1, nc.scalar)):
                xh = xpool.tile([128, 4 * D], F32, tag="xh", name=f"xh{h}")
                src_ap = node_full[h * 512:(h + 1) * 512, :].rearrange(
                    "(t j) d -> j t d", j=128)
                q.dma_start(out=xh.rearrange("j (t d) -> j t d", d=D),
                            in_=src_ap)
                xhalf.append(xh)
            xf_tiles = [xhalf[t // 4][:, (t % 4) * D:(t % 4 + 1) * D]
                        for t in range(8)]

            # ---------------- constants / views ----------------
            eps_t = singles.tile([128, 1], F32, tag="eps")
            nc.vector.memset(eps_t, LN_EPS)

            wl_sb = [cst[:, CW_WL[dc]:CW_WL[dc] + 16].bitcast(BF16)
                     for dc in range(2)]
            wr_sb = [cst[:, CW_WR[dc]:CW_WR[dc] + 16].bitcast(BF16)
                     for dc in range(2)]
            w2bf = cst[0:DH + 1, CW_W2:CW_W2 + 64].bitcast(BF16)
            mcf_sb = cst[:, CW_MCF:CW_MCF + 8]
            mcs_sb = cst[:, CW_MCS:CW_MCS + 1]
            bl_sb = cst[0:DH, CW_BL:CW_BL + 1]
            br_sb = cst[0:DH, CW_BR:CW_BR + 1]
            wl_row = crow[0:1, RW_WLR:RW_WLR + 16].bitcast(BF16)
            wr_row = crow[0:1, RW_WRR:RW_WRR + 16].bitcast(BF16)
            mrs_sb = crow[0:1, RW_MRS:RW_MRS + 64].bitcast(BF16)
            mrf_sb = [crow[0:1, RW_MRF[b]:RW_MRF[b] + 256].bitcast(BF16)
                      for b in range(B)]

            # ---------------- LayerNorm helper (-> bf16 copy) ---------------
            def layernorm_masked(x_t, mask_col_ap, name):
                """x_t [128, D] f32 -> new bf16 tile (x-mu)*rsqrt(var+eps)*mask."""
                st = stats.tile([128, 6], F32, tag="st")
                nc.vector.bn_stats(out=st, in_=x_t)
                mv = stats.tile([128, 2], F32, tag="mv")
                nc.vector.bn_aggr(out=mv, in_=st)
                sd = stats.tile([128, 1], F32, tag="sd")
                nc.scalar.activation(out=sd, in_=mv[:, 1:2],
                                     func=mybir.ActivationFunctionType.Sqrt,
                                     bias=eps_t, scale=1.0)
                rs = stats.tile([128, 1], F32, tag="rs")
                nc.vector.reciprocal(out=rs, in_=sd)
                rsm = stats.tile([128, 1], F32, tag="rsm")
                nc.vector.tensor_mul(out=rsm, in0=rs, in1=mask_col_ap)
                xb = xbpool.tile([128, D], BF16, tag="xb", name=name)
                nc.vector.tensor_scalar(out=xb, in0=x_t,
                                        scalar1=mv[:, 0:1], scalar2=rsm,
                                        op0=mybir.AluOpType.subtract,
                                        op1=mybir.AluOpType.mult)
                return xb

            # ---------------- shard path: leftT_all [33, B*LSH] bf16 --------
            xsb = layernorm_masked(xs, mcs_sb, "xsb")
            xsT = persist.tile([128, B * LSH * 2], BF16, tag="xsT")
            for dc in range(2):
                nc.sync.dma_start(out=xsT[:, dc * 128:(dc + 1) * 128],
                                  in_=xsb[:, dc * 128:(dc + 1) * 128],
                                  transpose=True)

            ps_l = ps_big.tile([128, 512], F32, tag="big",
                               name="ps_l")[0:DH, 0:B * LSH]
            for dc in range(2):
                nc.tensor.matmul(ps_l, wl_sb[dc],
                                 xsT[:, dc * 128:(dc + 1) * 128],
                                 start=(dc == 0), stop=False)
            nc.tensor.matmul(ps_l, wl_row, mrs_sb, start=False, stop=True)
            leftT = persist.tile([DH + 1, B * LSH], BF16, tag="leftT")
            nc.scalar.activation(out=leftT[0:DH, :], in_=ps_l,
                                 func=mybir.ActivationFunctionType.Identity,
                                 bias=bl_sb, scale=1.0)
            nc.vector.memset(leftT[DH:DH + 1, :], 1.0)

            # ---------------- full path (per batch): rightT[b] [33, L] bf16 -
            rightT = [persist.tile([DH + 1, L], BF16, tag=f"rt{b}",
                                   name=f"rt{b}") for b in range(B)]
            xT = [[persist.tile([128, L], BF16, tag=f"xT{b}_{dc}",
                                name=f"xT{b}_{dc}") for dc in range(2)]
                  for b in range(B)]

            def build_right(b):
                for lc in range(4):
                    t = b * 4 + lc
                    xb = layernorm_masked(xf_tiles[t], mcf_sb[:, t:t + 1],
                                          f"xb{t}")
                    for dc in range(2):
                        deng = nc.sync if (lc + dc) % 2 == 0 else nc.scalar
                        deng.dma_start(
                            out=xT[b][dc][:, lc * 128:(lc + 1) * 128],
                            in_=xb[:, dc * 128:(dc + 1) * 128],
                            transpose=True)
                    # project this j-chunk as soon as its columns exist
                    jc = lc
                    jsl = slice(jc * 128, (jc + 1) * 128)
                    prj = ps_big.tile([128, 512], F32, tag="big",
                                      name=f"prj{b}_{jc}")[0:DH, 0:128]
                    for dc in range(2):
                        nc.tensor.matmul(prj, wr_sb[dc],
                                         xT[b][dc][:, jsl],
                                         start=(dc == 0), stop=False)
                    nc.tensor.matmul(prj, wr_row, mrf_sb[b][:, jsl],
                                     start=False, stop=True)
                    nc.scalar.activation(out=rightT[b][0:DH, jsl],
                                         in_=prj,
                                         func=mybir.ActivationFunctionType.Identity,
                                         bias=br_sb, scale=1.0)
                nc.vector.memset(rightT[b][DH:DH + 1, :], 1.0)

            # ---------------- main pair loop ----------------
            def build_mp4(b, sg, il):
                """[33, 512] bf16 pack for 4 i's:
                mp[c, q*128 + p] = left[b, i(sg,il,q), c] * w2[c, p]."""
                mp = mp_pool.tile([DH + 1, 512], BF16, tag="mp",
                                  name=f"mp{b}_{sg}_{il}")
                col = b * LSH + (sg * 4 + il) * 4
                lsrc = leftT[:, col:col + 4].unsqueeze(-1).to_broadcast(
                    [DH + 1, 4, PAIR])
                wsrc = w2bf.unsqueeze(1).to_broadcast([DH + 1, 4, PAIR])
                dst = mp[:, :].rearrange("c (q p) -> c q p", p=PAIR)
                nc.gpsimd.tensor_mul(out=dst, in0=wsrc, in1=lsrc)
                return mp

            # all 32 M-packs depend only on leftT; hoist them in round order
            # on the (otherwise idle) GpSimd queue so no round ever waits.
            all_mps = {}

            def build_all_mps():
                for b in range(B):
                    for q2 in range(2):
                        for s in range(2):
                            for il in range(4):
                                all_mps[(b, q2, s, il)] = build_mp4(
                                    b, q2 * 2 + s, il)

            # drain engine pattern: DVE is slower per drain (658 vs 570ns),
            # give it 7 of every 16 (two j-chunks)
            DRAIN_DVE = {0: (0, 2, 4, 6), 1: (0, 3, 6)}

            def pair_rounds(b):
                for q2 in range(2):
                    mps = [[all_mps[(b, q2, s, il)] for il in range(4)]
                           for s in range(2)]
                    for jc in range(4):
                        lhsT = rightT[b][:, jc * 128:(jc + 1) * 128]
                        stg = stag_pool.tile([128, 2 * 16 * PAIR], BF16,
                                             tag="stag")
                        for s in range(2):
                            for il in range(4):
                                pb = ps_big.tile([128, 512], F32, tag="big")
                                nc.tensor.matmul(pb, lhsT, mps[s][il],
                                                 start=True, stop=True)
                                dst = stg[:, (s * 16 + il * 4) * PAIR:
                                          (s * 16 + il * 4 + 4) * PAIR]
                                if s * 4 + il in DRAIN_DVE[jc % 2]:
                                    nc.vector.tensor_copy(out=dst, in_=pb)
                                else:
                                    nc.scalar.copy(out=dst, in_=pb)
                        dst_ap = out[b, jc, q2, :, :, :, :]
                        src_ap = stg[:, :].rearrange("j (s i p) -> j s i p",
                                                     s=2, p=PAIR)
                        deng = nc.sync if b == 0 else nc.gpsimd
                        deng.dma_start(out=dst_ap, in_=src_ap)

            build_right(0)
            build_right(1)
            build_all_mps()
            pair_rounds(0)
            pair_rounds(1)

    nc.compile()
    names = ["node_full", "node_shard", "consts", "const_rows"]
    return nc, names


def _bfpack(a):
    """bf16 array [..., N] -> fp32-word-packed view [..., N//2]."""
    import ml_dtypes
    b = np.ascontiguousarray(np.asarray(a).astype(ml_dtypes.bfloat16))
    return b.view(np.float32)


def _prepare_in_maps(node, mask, ln_gamma, ln_beta, W_left, b_left, W_right,
                     b_right, W_out, b_out):
    f = np.float32
    node = np.ascontiguousarray(np.asarray(node, dtype=f))        # [B, L, D]
    mask_f = np.asarray(mask).astype(f)                           # [B, L]
    gamma = np.asarray(ln_gamma, dtype=f)
    beta = np.asarray(ln_beta, dtype=f)
    W_l = np.asarray(W_left, dtype=f)
    W_r = np.asarray(W_right, dtype=f)
    b_l = np.asarray(b_left, dtype=f)
    b_r = np.asarray(b_right, dtype=f)
    W_o = np.asarray(W_out, dtype=f)
    b_o = np.asarray(b_out, dtype=f)

    s = 1.0 / np.sqrt(np.float32(DH))
    w_left_e = np.concatenate([gamma[:, None] * W_l, (beta @ W_l)[None, :]], 0)
    w_right_e = np.concatenate([gamma[:, None] * W_r, (beta @ W_r)[None, :]],
                               0) * s
    w2 = np.concatenate([np.repeat(W_o, 2, axis=0), b_o[None, :]], 0)

    common_cst = np.zeros((128, NCONST), f)
    for dc in range(2):
        common_cst[:, CW_WL[dc]:CW_WL[dc] + 16] = \
            _bfpack(w_left_e[dc * 128:(dc + 1) * 128])
        common_cst[:, CW_WR[dc]:CW_WR[dc] + 16] = \
            _bfpack(w_right_e[dc * 128:(dc + 1) * 128])
    common_cst[0:DH + 1, CW_W2:CW_W2 + 64] = _bfpack(w2)
    common_cst[:, CW_MCF:CW_MCF + 8] = mask_f.reshape(-1, 128).T
    common_cst[0:DH, CW_BL] = b_l
    common_cst[0:DH, CW_BR] = b_r * s

    common_rows = np.zeros((1, NROWS), f)
    common_rows[0, RW_WLR:RW_WLR + 16] = _bfpack(w_left_e[D])
    common_rows[0, RW_WRR:RW_WRR + 16] = _bfpack(w_right_e[D])
    for b in range(B):
        common_rows[0, RW_MRF[b]:RW_MRF[b] + 256] = _bfpack(mask_f[b])

    node_flat = node.reshape(B * L, D)

    in_maps = []
    for c in range(NCORES):
        sl = slice(c * LSH, (c + 1) * LSH)
        shard = np.ascontiguousarray(node[:, sl, :].reshape(B * LSH, D))
        msk = mask_f[:, sl]                                       # [B, LSH]
        cstc = common_cst.copy()
        cstc[:, CW_MCS] = msk.reshape(-1)
        rowc = common_rows.copy()
        rowc[0, RW_MRS:RW_MRS + 64] = _bfpack(msk.reshape(-1))
        in_maps.append({
            "node_full": node_flat,
            "node_shard": shard,
            "consts": cstc,
            "const_rows": rowc,
        })
    return in_maps


def kernel(**inputs):
    global _COMPILED
    if _COMPILED is None:
        _COMPILED = _build_program()
    nc, names = _COMPILED
    in_maps = _prepare_in_maps(**inputs)
    res = run_bass_kernel_spmd(nc, in_maps, core_ids=list(range(NCORES)))
    full = np.empty((B, L, L, PAIR), np.float32)
    for c in range(NCORES):
        dev = res.results[c]["out"]   # [b, jc, q2, j, s, i16, p] bf16
        full[:, c * LSH:(c + 1) * LSH] = (
            dev.transpose(0, 2, 4, 5, 1, 3, 6)
               .reshape(B, LSH, L, PAIR).astype(np.float32))
    return full


if __name__ == "__main__":
    # self-test with NON-trivial gamma/beta/mask against a numpy reference
    rng = np.random.default_rng(1)
    mask = np.ones((B, L), dtype=bool)
    mask[0, 500:] = False        # exercise the mask path
    mask[1, :3] = False
    inputs = {
        "node": rng.standard_normal((B, L, D)).astype(np.float32),
        "mask": mask,
        "ln_gamma": (1.0 + 0.1 * rng.standard_normal(D)).astype(np.float32),
        "ln_beta": (0.1 * rng.standard_normal(D)).astype(np.float32),
        "W_left": (rng.standard_normal((D, DH)) / np.sqrt(D)).astype(np.float32),
        "b_left": (0.1 * rng.standard_normal(DH)).astype(np.float32),
        "W_right": (rng.standard_normal((D, DH)) / np.sqrt(D)).astype(np.float32),
        "b_right": (0.1 * rng.standard_normal(DH)).astype(np.float32),
        "W_out": (rng.standard_normal((H, PAIR)) / np.sqrt(H)).astype(np.float32),
        "b_out": (0.1 * rng.standard_normal(PAIR)).astype(np.float32),
    }

    def np_reference(node, mask, ln_gamma, ln_beta, W_left, b_left, W_right,
                     b_right, W_out, b_out):
        node = node.astype(np.float64)
        mu = node.mean(-1, keepdims=True)
        var = ((node - mu) ** 2).mean(-1, keepdims=True)
        x = (node - mu) / np.sqrt(var + LN_EPS) * ln_gamma + ln_beta
        x = x * mask[..., None]
        left = (x @ W_left + b_left).reshape(B, L, H, -1)
        right = ((x @ W_right + b_right) / np.sqrt(DH)).reshape(B, L, H, -1)
        o = np.einsum("bihk,bjhk->bijh", left, right)
        return np.einsum("bijh,hp->bijp", o, W_out) + b_out

    got = kernel(**inputs)
    exp = np_reference(**inputs)
    rel = np.abs(got - exp).max() / np.abs(exp).max()
    print("general-path rel err:", rel)
    assert rel < 1.8e-2, rel
    print("OK", got.shape, got.dtype)


# revision 23
# speedup vs baseline: 1.2769x; 1.0328x over previous
"""Trainium2 Bass kernel for nn_Node2Pair_bias (LayerNorm -> dual projection ->
pair outer-product -> head-mix linear).

Reference computation (B=2, L=512, D=256, DH=32, H=16, K=2, P=128):
    x   = LayerNorm(node) * gamma + beta, masked        [B, L, D]
    left  = (x @ W_left + b_left)                       [B, L, DH] -> [B,L,H,K]
    right = ((x @ W_right + b_right)/sqrt(DH))          [B, L, DH] -> [B,L,H,K]
    out[b,i,j,h] = sum_k left[b,i,h,k]*right[b,j,h,k]
    out[b,i,j,p] = sum_h out[b,i,j,h]*W_out[h,p] + b_out[p]   [B, L, L, P]

Mathematical restructuring (c = (h,k) combined channel, 0..31):
    out[b,i,j,p] = sum_c right[b,j,c] * (left[b,i,c] * W2[c,p]) + b_out[p]
with W2[c,p] = W_out[c//2, p].  M-packs M[c, (q,p)] = left[b,i_q,c]*W2[c,p]
for 4 i's are built on GpSimd with one broadcast multiply each; the pair
matmul is bf16 x bf16 -> fp32 PSUM:  lhsT=rightT[33, j-chunk 128] x
rhs=M_pack[33, 512].  Row 32 of rightT is constant 1 and row 32 of the
M-pack is b_out, which adds the bias inside the same matmul.

The PE on this part streams at ~1.2 GHz (427ns per 512-col matmul,
measured), so PE column count is the kernel's hard floor; everything else
is arranged to keep the PE streaming continuously:
  - all x transposes are done by the DMA xbar (bf16 dma_start transpose)
    instead of the PE,
  - projections run in bf16 (1 cycle/row instead of 4),
  - PSUM is a single 8 x 1-bank pool so the matmul/drain ring never
    couples (one matmul per bank, 8 in flight),
  - drains [128,512] f32->bf16 alternate DVE(56)/ACT(72) by measured rate,
  - output DMA triggers stay off the drain queues: sync ring for b=0,
    gpsimd (SWDGE) for b=1 (queued after all M-packs).

Weights/masks ship as bf16 pairs packed in fp32 DRAM words, viewed on-chip
via AP.bitcast.  The host converts the bf16 output back to fp32 while
assembling.

Sharding: the i axis of L is split across the 8 cores (sequence-parallel);
each core holds its [B, 64] slice of `left` plus the full `right` side and
writes a [B, 64, L, P] output shard.  No cross-device communication.

LayerNorm gamma/beta are folded into the projection weights on the host
(exact algebra): W_e = gamma[:,None]*W, with an extra K=1 accumulation row
carrying beta@W * mask.
"""

import os
import sys

sys.path.insert(0, "/opt/trn_rl_repo")

import numpy as np

import concourse.bass as bass
import concourse.mybir as mybir
import concourse.tile as tile
from concourse import bacc
from concourse.bass_utils import run_bass_kernel_spmd
from concourse.masks import make_identity

F32 = mybir.dt.float32
BF16 = mybir.dt.bfloat16

B, L, D = 2, 512, 256
DH, H, PAIR = 32, 16, 128
NCORES = 8
LSH = L // NCORES          # 64 i's per core per batch
LN_EPS = 1e-5

# packed-constant column maps.  bf16 payloads are packed pairwise into f32
# words (host .view(np.float32)); offsets below are in f32 columns.
CW_WL = (0, 16)            # [128, 32]bf16 x2: gamma*W_l rows 0-127 / 128-255
CW_WR = (32, 48)
CW_W2 = 64                 # [33, 128]bf16 (W_out rows repeated x2 + b_out)
CW_MCF = 128               # [128, 8] f32 LN column masks (full seq)
CW_MCS = 136               # [128, 1] f32 (shard)
CW_BL = 137                # [32, 1] f32
CW_BR = 138                # [32, 1] f32
NCONST = 139
RW_WLR = 0                 # [1, 32]bf16  row 256 of w_left_e
RW_WRR = 16
RW_MRS = 32                # [1, 128]bf16 shard mask row
RW_MRF = (96, 352)         # [1, 512]bf16 full mask rows x2
NROWS = 608

_COMPILED = None  # (nc, input_names)


def _build_program():
    nc = bacc.Bacc("TRN2", target_bir_lowering=False, debug=False,
                   num_devices=NCORES)

    node_full = nc.dram_tensor("node_full", [B * L, D], F32,
                               kind="ExternalInput").ap()
    node_shard = nc.dram_tensor("node_shard", [B * LSH, D], F32,
                                kind="ExternalInput").ap()
    consts = nc.dram_tensor("consts", [128, NCONST], F32,
                            kind="ExternalInput").ap()
    const_rows = nc.dram_tensor("const_rows", [1, NROWS], F32,
                                kind="ExternalInput").ap()

    # Permuted output layout: [b, jc, q2, j, s, i16, p] (bf16) — each staging
    # buffer lands as one fully contiguous 1 MiB stream (8 KiB per partition
    # run).  sg = q2*2 + s; i_local = sg*16 + i16.  The host un-permutes +
    # upcasts while assembling the full output.
    out = nc.dram_tensor("out", [B, 4, 2, 2, 128, 16, PAIR], BF16,
                         kind="ExternalOutput").ap()

    with tile.TileContext(nc) as tc:
        with (
            tc.tile_pool(name="singles", bufs=1) as singles,
            tc.tile_pool(name="xpool", bufs=2) as xpool,
            tc.tile_pool(name="xbpool", bufs=4) as xbpool,
            tc.tile_pool(name="stats", bufs=4) as stats,
            tc.tile_pool(name="persist", bufs=1) as persist,
            tc.tile_pool(name="mp", bufs=32) as mp_pool,
            tc.tile_pool(name="stag", bufs=8) as stag_pool,
            tc.tile_pool(name="ps_big", bufs=8, space="PSUM") as ps_big,
        ):
            # ---------------- input loads (2 HWDGE rings, batched) ----------
            xs = xpool.tile([128, D], F32, tag="x", name="xs")
            nc.sync.dma_start(out=xs, in_=node_shard[:, :])
            cst = singles.tile([128, NCONST], F32, tag="cst")
            nc.scalar.dma_start(out=cst, in_=consts[:, :])
            crow = singles.tile([1, NROWS], F32, tag="crow")
            nc.scalar.dma_start(out=crow, in_=const_rows[:, :])
            xhalf = []
            for h, q in ((0, nc.sync), (1, nc.scalar)):
                xh = xpool.tile([128, 4 * D], F32, tag="xh", name=f"xh{h}")
                src_ap = node_full[h * 512:(h + 1) * 512, :].rearrange(
                    "(t j) d -> j t d", j=128)
                q.dma_start(out=xh.rearrange("j (t d) -> j t d", d=D),
                            in_=src_ap)
                xhalf.append(xh)
            xf_tiles = [xhalf[t // 4][:, (t % 4) * D:(t % 4 + 1) * D]
                        for t in range(8)]

            # ---------------- constants / views ----------------
            eps_t = singles.tile([128, 1], F32, tag="eps")
            nc.vector.memset(eps_t, LN_EPS)
            ident_f = singles.tile([128, 128], F32, tag="identf")
            make_identity(nc, ident_f)
            ident_bf = singles.tile([128, 128], BF16, tag="identbf")
            nc.vector.tensor_copy(out=ident_bf, in_=ident_f)

            wl_sb = [cst[:, CW_WL[dc]:CW_WL[dc] + 16].bitcast(BF16)
                     for dc in range(2)]
            wr_sb = [cst[:, CW_WR[dc]:CW_WR[dc] + 16].bitcast(BF16)
                     for dc in range(2)]
            w2bf = cst[0:DH + 1, CW_W2:CW_W2 + 64].bitcast(BF16)
            mcf_sb = cst[:, CW_MCF:CW_MCF + 8]
            mcs_sb = cst[:, CW_MCS:CW_MCS + 1]
            bl_sb = cst[0:DH, CW_BL:CW_BL + 1]
            br_sb = cst[0:DH, CW_BR:CW_BR + 1]
            wl_row = crow[0:1, RW_WLR:RW_WLR + 16].bitcast(BF16)
            wr_row = crow[0:1, RW_WRR:RW_WRR + 16].bitcast(BF16)
            mrs_sb = crow[0:1, RW_MRS:RW_MRS + 64].bitcast(BF16)
            mrf_sb = [crow[0:1, RW_MRF[b]:RW_MRF[b] + 256].bitcast(BF16)
                      for b in range(B)]

            # ---------------- LayerNorm helper (-> bf16 copy) ---------------
            def layernorm_masked(x_t, mask_col_ap, name):
                """x_t [128, D] f32 -> new bf16 tile (x-mu)*rsqrt(var+eps)*mask."""
                st = stats.tile([128, 6], F32, tag="st")
                nc.vector.bn_stats(out=st, in_=x_t)
                mv = stats.tile([128, 2], F32, tag="mv")
                nc.vector.bn_aggr(out=mv, in_=st)
                sd = stats.tile([128, 1], F32, tag="sd")
                nc.scalar.activation(out=sd, in_=mv[:, 1:2],
                                     func=mybir.ActivationFunctionType.Sqrt,
                                     bias=eps_t, scale=1.0)
                rs = stats.tile([128, 1], F32, tag="rs")
                nc.vector.reciprocal(out=rs, in_=sd)
                rsm = stats.tile([128, 1], F32, tag="rsm")
                nc.vector.tensor_mul(out=rsm, in0=rs, in1=mask_col_ap)
                xb = xbpool.tile([128, D], BF16, tag="xb", name=name)
                nc.vector.tensor_scalar(out=xb, in0=x_t,
                                        scalar1=mv[:, 0:1], scalar2=rsm,
                                        op0=mybir.AluOpType.subtract,
                                        op1=mybir.AluOpType.mult)
                return xb

            # ---------------- shard path: leftT_all [33, B*LSH] bf16 --------
            xsb = layernorm_masked(xs, mcs_sb, "xsb")
            xsT = persist.tile([128, B * LSH * 2], BF16, tag="xsT")
            for dc in range(2):
                pt = ps_big.tile([128, 512], F32, tag="big",
                                 name=f"tps{dc}")[:, 0:64].bitcast(BF16)
                nc.tensor.transpose(pt, xsb[:, dc * 128:(dc + 1) * 128],
                                    ident_bf)
                if dc == 0:
                    nc.vector.tensor_copy(
                        out=xsT[:, dc * 128:(dc + 1) * 128], in_=pt)
                else:
                    nc.scalar.copy(
                        out=xsT[:, dc * 128:(dc + 1) * 128], in_=pt)

            ps_l = ps_big.tile([128, 512], F32, tag="big",
                               name="ps_l")[0:DH, 0:B * LSH]
            for dc in range(2):
                nc.tensor.matmul(ps_l, wl_sb[dc],
                                 xsT[:, dc * 128:(dc + 1) * 128],
                                 start=(dc == 0), stop=False)
            nc.tensor.matmul(ps_l, wl_row, mrs_sb, start=False, stop=True)
            leftT = persist.tile([DH + 1, B * LSH], BF16, tag="leftT")
            nc.scalar.activation(out=leftT[0:DH, :], in_=ps_l,
                                 func=mybir.ActivationFunctionType.Identity,
                                 bias=bl_sb, scale=1.0)
            nc.vector.memset(leftT[DH:DH + 1, :], 1.0)

            # ---------------- full path (per batch): rightT[b] [33, L] bf16 -
            rightT = [persist.tile([DH + 1, L], BF16, tag=f"rt{b}",
                                   name=f"rt{b}") for b in range(B)]
            xT = [[persist.tile([128, L], BF16, tag=f"xT{b}_{dc}",
                                name=f"xT{b}_{dc}") for dc in range(2)]
                  for b in range(B)]

            def build_right(b):
                for lc in range(4):
                    t = b * 4 + lc
                    xb = layernorm_masked(xf_tiles[t], mcf_sb[:, t:t + 1],
                                          f"xb{t}")
                    for dc in range(2):
                        deng = nc.sync if (lc + dc) % 2 == 0 else nc.scalar
                        deng.dma_start(
                            out=xT[b][dc][:, lc * 128:(lc + 1) * 128],
                            in_=xb[:, dc * 128:(dc + 1) * 128],
                            transpose=True)
                    # project this j-chunk as soon as its columns exist
                    jc = lc
                    jsl = slice(jc * 128, (jc + 1) * 128)
                    prj = ps_big.tile([128, 512], F32, tag="big",
                                      name=f"prj{b}_{jc}")[0:DH, 0:128]
                    for dc in range(2):
                        nc.tensor.matmul(prj, wr_sb[dc],
                                         xT[b][dc][:, jsl],
                                         start=(dc == 0), stop=False)
                    nc.tensor.matmul(prj, wr_row, mrf_sb[b][:, jsl],
                                     start=False, stop=True)
                    nc.scalar.activation(out=rightT[b][0:DH, jsl],
                                         in_=prj,
                                         func=mybir.ActivationFunctionType.Identity,
                                         bias=br_sb, scale=1.0)
                nc.vector.memset(rightT[b][DH:DH + 1, :], 1.0)

            # ---------------- main pair loop ----------------
            def build_mp4(b, sg, il):
                """[33, 512] bf16 pack for 4 i's:
                mp[c, q*128 + p] = left[b, i(sg,il,q), c] * w2[c, p]."""
                mp = mp_pool.tile([DH + 1, 512], BF16, tag="mp",
                                  name=f"mp{b}_{sg}_{il}")
                col = b * LSH + (sg * 4 + il) * 4
                lsrc = leftT[:, col:col + 4].unsqueeze(-1).to_broadcast(
                    [DH + 1, 4, PAIR])
                wsrc = w2bf.unsqueeze(1).to_broadcast([DH + 1, 4, PAIR])
                dst = mp[:, :].rearrange("c (q p) -> c q p", p=PAIR)
                nc.gpsimd.tensor_mul(out=dst, in0=wsrc, in1=lsrc)
                return mp

            # all 32 M-packs depend only on leftT; hoist them in round order
            # on the (otherwise idle) GpSimd queue so no round ever waits.
            all_mps = {}

            def build_all_mps():
                for b in range(B):
                    for q2 in range(2):
                        for s in range(2):
                            for il in range(4):
                                all_mps[(b, q2, s, il)] = build_mp4(
                                    b, q2 * 2 + s, il)

            # strict DVE/ACT drain alternation (balanced 64/64)
            DRAIN_DVE = {0: (0, 2), 1: (1, 3)}

            def pair_rounds(b):
                for q2 in range(2):
                    mps = [[all_mps[(b, q2, s, il)] for il in range(4)]
                           for s in range(2)]
                    for jc in range(4):
                        lhsT = rightT[b][:, jc * 128:(jc + 1) * 128]
                        stg = stag_pool.tile([128, 2 * 16 * PAIR], BF16,
                                             tag="stag")
                        deng = nc.sync if b == 0 else nc.gpsimd
                        for s in range(2):
                            for il in range(4):
                                pb = ps_big.tile([128, 512], F32, tag="big")
                                nc.tensor.matmul(pb, lhsT, mps[s][il],
                                                 start=True, stop=True)
                                dst = stg[:, (s * 16 + il * 4) * PAIR:
                                          (s * 16 + il * 4 + 4) * PAIR]
                                if il in DRAIN_DVE[jc % 2]:
                                    nc.vector.tensor_copy(out=dst, in_=pb)
                                else:
                                    nc.scalar.copy(out=dst, in_=pb)
                            # fire this s-half as soon as its 4 drains land
                            dst_ap = out[b, jc, q2, s, :, :, :]
                            src_ap = stg[:, s * 16 * PAIR:
                                         (s + 1) * 16 * PAIR].rearrange(
                                "j (i p) -> j i p", p=PAIR)
                            deng.dma_start(out=dst_ap, in_=src_ap)

            build_right(0)
            build_right(1)
            build_all_mps()
            pair_rounds(0)
            pair_rounds(1)

    nc.compile()
    names = ["node_full", "node_shard", "consts", "const_rows"]
    return nc, names


def _bfpack(a):
    """bf16 array [..., N] -> fp32-word-packed view [..., N//2]."""
    import ml_dtypes
    b = np.ascontiguousarray(np.asarray(a).astype(ml_dtypes.bfloat16))
    return b.view(np.float32)


def _prepare_in_maps(node, mask, ln_gamma, ln_beta, W_left, b_left, W_right,
                     b_right, W_out, b_out):
    f = np.float32
    node = np.ascontiguousarray(np.asarray(node, dtype=f))        # [B, L, D]
    mask_f = np.asarray(mask).astype(f)                           # [B, L]
    gamma = np.asarray(ln_gamma, dtype=f)
    beta = np.asarray(ln_beta, dtype=f)
    W_l = np.asarray(W_left, dtype=f)
    W_r = np.asarray(W_right, dtype=f)
    b_l = np.asarray(b_left, dtype=f)
    b_r = np.asarray(b_right, dtype=f)
    W_o = np.asarray(W_out, dtype=f)
    b_o = np.asarray(b_out, dtype=f)

    s = 1.0 / np.sqrt(np.float32(DH))
    w_left_e = np.concatenate([gamma[:, None] * W_l, (beta @ W_l)[None, :]], 0)
    w_right_e = np.concatenate([gamma[:, None] * W_r, (beta @ W_r)[None, :]],
                               0) * s
    w2 = np.concatenate([np.repeat(W_o, 2, axis=0), b_o[None, :]], 0)

    common_cst = np.zeros((128, NCONST), f)
    for dc in range(2):
        common_cst[:, CW_WL[dc]:CW_WL[dc] + 16] = \
            _bfpack(w_left_e[dc * 128:(dc + 1) * 128])
        common_cst[:, CW_WR[dc]:CW_WR[dc] + 16] = \
            _bfpack(w_right_e[dc * 128:(dc + 1) * 128])
    common_cst[0:DH + 1, CW_W2:CW_W2 + 64] = _bfpack(w2)
    common_cst[:, CW_MCF:CW_MCF + 8] = mask_f.reshape(-1, 128).T
    common_cst[0:DH, CW_BL] = b_l
    common_cst[0:DH, CW_BR] = b_r * s

    common_rows = np.zeros((1, NROWS), f)
    common_rows[0, RW_WLR:RW_WLR + 16] = _bfpack(w_left_e[D])
    common_rows[0, RW_WRR:RW_WRR + 16] = _bfpack(w_right_e[D])
    for b in range(B):
        common_rows[0, RW_MRF[b]:RW_MRF[b] + 256] = _bfpack(mask_f[b])

    node_flat = node.reshape(B * L, D)

    in_maps = []
    for c in range(NCORES):
        sl = slice(c * LSH, (c + 1) * LSH)
        shard = np.ascontiguousarray(node[:, sl, :].reshape(B * LSH, D))
        msk = mask_f[:, sl]                                       # [B, LSH]
        cstc = common_cst.copy()
        cstc[:, CW_MCS] = msk.reshape(-1)
        rowc = common_rows.copy()
        rowc[0, RW_MRS:RW_MRS + 64] = _bfpack(msk.reshape(-1))
        in_maps.append({
            "node_full": node_flat,
            "node_shard": shard,
            "consts": cstc,
            "const_rows": rowc,
        })
    return in_maps


def kernel(**inputs):
    global _COMPILED
    if _COMPILED is None:
        _COMPILED = _build_program()
    nc, names = _COMPILED
    in_maps = _prepare_in_maps(**inputs)
    res = run_bass_kernel_spmd(nc, in_maps, core_ids=list(range(NCORES)))
    full = np.empty((B, L, L, PAIR), np.float32)
    for c in range(NCORES):
        dev = res.results[c]["out"]   # [b, jc, q2, s, j, i16, p] bf16
        full[:, c * LSH:(c + 1) * LSH] = (
            dev.transpose(0, 2, 3, 5, 1, 4, 6)
               .reshape(B, LSH, L, PAIR).astype(np.float32))
    return full


if __name__ == "__main__":
    # self-test with NON-trivial gamma/beta/mask against a numpy reference
    rng = np.random.default_rng(1)
    mask = np.ones((B, L), dtype=bool)
    mask[0, 500:] = False        # exercise the mask path
    mask[1, :3] = False
    inputs = {
        "node": rng.standard_normal((B, L, D)).astype(np.float32),
        "mask": mask,
        "ln_gamma": (1.0 + 0.1 * rng.standard_normal(D)).astype(np.float32),
        "ln_beta": (0.1 * rng.standard_normal(D)).astype(np.float32),
        "W_left": (rng.standard_normal((D, DH)) / np.sqrt(D)).astype(np.float32),
        "b_left": (0.1 * rng.standard_normal(DH)).astype(np.float32),
        "W_right": (rng.standard_normal((D, DH)) / np.sqrt(D)).astype(np.float32),
        "b_right": (0.1 * rng.standard_normal(DH)).astype(np.float32),
        "W_out": (rng.standard_normal((H, PAIR)) / np.sqrt(H)).astype(np.float32),
        "b_out": (0.1 * rng.standard_normal(PAIR)).astype(np.float32),
    }

    def np_reference(node, mask, ln_gamma, ln_beta, W_left, b_left, W_right,
                     b_right, W_out, b_out):
        node = node.astype(np.float64)
        mu = node.mean(-1, keepdims=True)
        var = ((node - mu) ** 2).mean(-1, keepdims=True)
        x = (node - mu) / np.sqrt(var + LN_EPS) * ln_gamma + ln_beta
        x = x * mask[..., None]
        left = (x @ W_left + b_left).reshape(B, L, H, -1)
        right = ((x @ W_right + b_right) / np.sqrt(DH)).reshape(B, L, H, -1)
        o = np.einsum("bihk,bjhk->bijh", left, right)
        return np.einsum("bijh,hp->bijp", o, W_out) + b_out

    got = kernel(**inputs)
    exp = np_reference(**inputs)
    rel = np.abs(got - exp).max() / np.abs(exp).max()
    print("general-path rel err:", rel)
    assert rel < 1.8e-2, rel
    print("OK", got.shape, got.dtype)


# revision 24
# speedup vs baseline: 1.2928x; 1.0125x over previous
"""Trainium2 Bass kernel for nn_Node2Pair_bias (LayerNorm -> dual projection ->
pair outer-product -> head-mix linear).

Reference computation (B=2, L=512, D=256, DH=32, H=16, K=2, P=128):
    x   = LayerNorm(node) * gamma + beta, masked        [B, L, D]
    left  = (x @ W_left + b_left)                       [B, L, DH] -> [B,L,H,K]
    right = ((x @ W_right + b_right)/sqrt(DH))          [B, L, DH] -> [B,L,H,K]
    out[b,i,j,h] = sum_k left[b,i,h,k]*right[b,j,h,k]
    out[b,i,j,p] = sum_h out[b,i,j,h]*W_out[h,p] + b_out[p]   [B, L, L, P]

Mathematical restructuring (c = (h,k) combined channel, 0..31):
    out[b,i,j,p] = sum_c right[b,j,c] * (left[b,i,c] * W2[c,p]) + b_out[p]
with W2[c,p] = W_out[c//2, p].  M-packs M[c, (q,p)] = left[b,i_q,c]*W2[c,p]
for 4 i's are built on GpSimd with one broadcast multiply each; the pair
matmul is bf16 x bf16 -> fp32 PSUM:  lhsT=rightT[33, j-chunk 128] x
rhs=M_pack[33, 512].  Row 32 of rightT is constant 1 and row 32 of the
M-pack is b_out, which adds the bias inside the same matmul.

The PE on this part streams at ~1.2 GHz (427ns per 512-col matmul,
measured), so PE column count is the kernel's hard floor; everything else
is arranged to keep the PE streaming continuously:
  - all x transposes are done by the DMA xbar (bf16 dma_start transpose)
    instead of the PE,
  - projections run in bf16 (1 cycle/row instead of 4),
  - PSUM is a single 8 x 1-bank pool so the matmul/drain ring never
    couples (one matmul per bank, 8 in flight),
  - drains [128,512] f32->bf16 alternate DVE(56)/ACT(72) by measured rate,
  - output DMA triggers stay off the drain queues: sync ring for b=0,
    gpsimd (SWDGE) for b=1 (queued after all M-packs).

Weights/masks ship as bf16 pairs packed in fp32 DRAM words, viewed on-chip
via AP.bitcast.  The host converts the bf16 output back to fp32 while
assembling.

Sharding: the i axis of L is split across the 8 cores (sequence-parallel);
each core holds its [B, 64] slice of `left` plus the full `right` side and
writes a [B, 64, L, P] output shard.  No cross-device communication.

LayerNorm gamma/beta are folded into the projection weights on the host
(exact algebra): W_e = gamma[:,None]*W, with an extra K=1 accumulation row
carrying beta@W * mask.
"""

import os
import sys

sys.path.insert(0, "/opt/trn_rl_repo")

import numpy as np

import concourse.bass as bass
import concourse.mybir as mybir
import concourse.tile as tile
from concourse import bacc
from concourse.bass_utils import run_bass_kernel_spmd
from concourse.masks import make_identity

F32 = mybir.dt.float32
BF16 = mybir.dt.bfloat16

B, L, D = 2, 512, 256
DH, H, PAIR = 32, 16, 128
NCORES = 8
LSH = L // NCORES          # 64 i's per core per batch
LN_EPS = 1e-5

# packed-constant column maps.  bf16 payloads are packed pairwise into f32
# words (host .view(np.float32)); offsets below are in f32 columns.
CW_WL = (0, 16)            # [128, 32]bf16 x2: gamma*W_l rows 0-127 / 128-255
CW_WR = (32, 48)
CW_W2 = 64                 # [33, 128]bf16 (W_out rows repeated x2 + b_out)
CW_MCF = 128               # [128, 8] f32 LN column masks (full seq)
CW_MCS = 136               # [128, 1] f32 (shard)
CW_BL = 137                # [32, 1] f32
CW_BR = 138                # [32, 1] f32
NCONST = 139
RW_WLR = 0                 # [1, 32]bf16  row 256 of w_left_e
RW_WRR = 16
RW_MRS = 32                # [1, 128]bf16 shard mask row
RW_MRF = (96, 352)         # [1, 512]bf16 full mask rows x2
NROWS = 608

_COMPILED = None  # (nc, input_names)


def _build_program():
    nc = bacc.Bacc("TRN2", target_bir_lowering=False, debug=False,
                   num_devices=NCORES)

    node_full = nc.dram_tensor("node_full", [B * L, D], F32,
                               kind="ExternalInput").ap()
    node_shard = nc.dram_tensor("node_shard", [B * LSH, D], F32,
                                kind="ExternalInput").ap()
    consts = nc.dram_tensor("consts", [128, NCONST], F32,
                            kind="ExternalInput").ap()
    const_rows = nc.dram_tensor("const_rows", [1, NROWS], F32,
                                kind="ExternalInput").ap()

    # Permuted output layout: [b, jc, q2, j, s, i16, p] (bf16) — each staging
    # buffer lands as one fully contiguous 1 MiB stream (8 KiB per partition
    # run).  sg = q2*2 + s; i_local = sg*16 + i16.  The host un-permutes +
    # upcasts while assembling the full output.
    out = nc.dram_tensor("out", [B, 4, 2, 2, 128, 16, PAIR], BF16,
                         kind="ExternalOutput").ap()

    with tile.TileContext(nc) as tc:
        with (
            tc.tile_pool(name="singles", bufs=1) as singles,
            tc.tile_pool(name="xpool", bufs=2) as xpool,
            tc.tile_pool(name="xbpool", bufs=4) as xbpool,
            tc.tile_pool(name="stats", bufs=4) as stats,
            tc.tile_pool(name="persist", bufs=1) as persist,
            tc.tile_pool(name="mp", bufs=32) as mp_pool,
            tc.tile_pool(name="stag", bufs=8) as stag_pool,
            tc.tile_pool(name="ps_big", bufs=8, space="PSUM") as ps_big,
        ):
            # ---------------- input loads (2 HWDGE rings, batched) ----------
            xs = xpool.tile([128, D], F32, tag="x", name="xs")
            nc.sync.dma_start(out=xs, in_=node_shard[:, :])
            cst = singles.tile([128, NCONST], F32, tag="cst")
            nc.scalar.dma_start(out=cst, in_=consts[:, :])
            crow = singles.tile([1, NROWS], F32, tag="crow")
            nc.scalar.dma_start(out=crow, in_=const_rows[:, :])
            xhalf = []
            for h, q in ((0, nc.sync), (1, nc.scalar)):
                xh = xpool.tile([128, 4 * D], F32, tag="xh", name=f"xh{h}")
                src_ap = node_full[h * 512:(h + 1) * 512, :].rearrange(
                    "(t j) d -> j t d", j=128)
                q.dma_start(out=xh.rearrange("j (t d) -> j t d", d=D),
                            in_=src_ap)
                xhalf.append(xh)
            xf_tiles = [xhalf[t // 4][:, (t % 4) * D:(t % 4 + 1) * D]
                        for t in range(8)]

            # ---------------- constants / views ----------------
            eps_t = singles.tile([128, 1], F32, tag="eps")
            nc.vector.memset(eps_t, LN_EPS)
            ident_f = singles.tile([128, 128], F32, tag="identf")
            make_identity(nc, ident_f)
            ident_bf = singles.tile([128, 128], BF16, tag="identbf")
            nc.vector.tensor_copy(out=ident_bf, in_=ident_f)

            wl_sb = [cst[:, CW_WL[dc]:CW_WL[dc] + 16].bitcast(BF16)
                     for dc in range(2)]
            wr_sb = [cst[:, CW_WR[dc]:CW_WR[dc] + 16].bitcast(BF16)
                     for dc in range(2)]
            w2bf = cst[0:DH + 1, CW_W2:CW_W2 + 64].bitcast(BF16)
            mcf_sb = cst[:, CW_MCF:CW_MCF + 8]
            mcs_sb = cst[:, CW_MCS:CW_MCS + 1]
            bl_sb = cst[0:DH, CW_BL:CW_BL + 1]
            br_sb = cst[0:DH, CW_BR:CW_BR + 1]
            wl_row = crow[0:1, RW_WLR:RW_WLR + 16].bitcast(BF16)
            wr_row = crow[0:1, RW_WRR:RW_WRR + 16].bitcast(BF16)
            mrs_sb = crow[0:1, RW_MRS:RW_MRS + 64].bitcast(BF16)
            mrf_sb = [crow[0:1, RW_MRF[b]:RW_MRF[b] + 256].bitcast(BF16)
                      for b in range(B)]

            # ---------------- LayerNorm helper (-> bf16 copy) ---------------
            def layernorm_masked(x_t, mask_col_ap, name):
                """x_t [128, D] f32 -> new bf16 tile (x-mu)*rsqrt(var+eps)*mask."""
                st = stats.tile([128, 6], F32, tag="st")
                nc.vector.bn_stats(out=st, in_=x_t)
                mv = stats.tile([128, 2], F32, tag="mv")
                nc.vector.bn_aggr(out=mv, in_=st)
                sd = stats.tile([128, 1], F32, tag="sd")
                nc.scalar.activation(out=sd, in_=mv[:, 1:2],
                                     func=mybir.ActivationFunctionType.Sqrt,
                                     bias=eps_t, scale=1.0)
                rs = stats.tile([128, 1], F32, tag="rs")
                nc.vector.reciprocal(out=rs, in_=sd)
                rsm = stats.tile([128, 1], F32, tag="rsm")
                nc.vector.tensor_mul(out=rsm, in0=rs, in1=mask_col_ap)
                xb = xbpool.tile([128, D], BF16, tag="xb", name=name)
                nc.vector.tensor_scalar(out=xb, in0=x_t,
                                        scalar1=mv[:, 0:1], scalar2=rsm,
                                        op0=mybir.AluOpType.subtract,
                                        op1=mybir.AluOpType.mult)
                return xb

            # ---------------- shard path: leftT_all [33, B*LSH] bf16 --------
            xsb = layernorm_masked(xs, mcs_sb, "xsb")
            xsT = persist.tile([128, B * LSH * 2], BF16, tag="xsT")
            for dc in range(2):
                pt = ps_big.tile([128, 512], F32, tag="big",
                                 name=f"tps{dc}")[:, 0:64].bitcast(BF16)
                nc.tensor.transpose(pt, xsb[:, dc * 128:(dc + 1) * 128],
                                    ident_bf)
                if dc == 0:
                    nc.vector.tensor_copy(
                        out=xsT[:, dc * 128:(dc + 1) * 128], in_=pt)
                else:
                    nc.scalar.copy(
                        out=xsT[:, dc * 128:(dc + 1) * 128], in_=pt)

            ps_l = ps_big.tile([128, 512], F32, tag="big",
                               name="ps_l")[0:DH, 0:B * LSH]
            for dc in range(2):
                nc.tensor.matmul(ps_l, wl_sb[dc],
                                 xsT[:, dc * 128:(dc + 1) * 128],
                                 start=(dc == 0), stop=False)
            nc.tensor.matmul(ps_l, wl_row, mrs_sb, start=False, stop=True)
            leftT = persist.tile([DH + 1, B * LSH], BF16, tag="leftT")
            nc.scalar.activation(out=leftT[0:DH, :], in_=ps_l,
                                 func=mybir.ActivationFunctionType.Identity,
                                 bias=bl_sb, scale=1.0)
            nc.vector.memset(leftT[DH:DH + 1, :], 1.0)

            # ---------------- full path (per batch): rightT[b] [33, L] bf16 -
            rightT = [persist.tile([DH + 1, L], BF16, tag=f"rt{b}",
                                   name=f"rt{b}") for b in range(B)]
            xT = [[persist.tile([128, L], BF16, tag=f"xT{b}_{dc}",
                                name=f"xT{b}_{dc}") for dc in range(2)]
                  for b in range(B)]

            def build_right(b):
                for lc in range(4):
                    t = b * 4 + lc
                    xb = layernorm_masked(xf_tiles[t], mcf_sb[:, t:t + 1],
                                          f"xb{t}")
                    for dc in range(2):
                        pt = ps_big.tile([128, 512], F32, tag="big",
                                         name=f"tp{b}_{lc}_{dc}"
                                         )[:, 0:64].bitcast(BF16)
                        nc.tensor.transpose(
                            pt, xb[:, dc * 128:(dc + 1) * 128], ident_bf)
                        dst = xT[b][dc][:, lc * 128:(lc + 1) * 128]
                        if (lc + dc) % 2 == 0:
                            nc.vector.tensor_copy(out=dst, in_=pt)
                        else:
                            nc.scalar.copy(out=dst, in_=pt)
                    # project this j-chunk as soon as its columns exist
                    jc = lc
                    jsl = slice(jc * 128, (jc + 1) * 128)
                    prj = ps_big.tile([128, 512], F32, tag="big",
                                      name=f"prj{b}_{jc}")[0:DH, 0:128]
                    for dc in range(2):
                        nc.tensor.matmul(prj, wr_sb[dc],
                                         xT[b][dc][:, jsl],
                                         start=(dc == 0), stop=False)
                    nc.tensor.matmul(prj, wr_row, mrf_sb[b][:, jsl],
                                     start=False, stop=True)
                    nc.scalar.activation(out=rightT[b][0:DH, jsl],
                                         in_=prj,
                                         func=mybir.ActivationFunctionType.Identity,
                                         bias=br_sb, scale=1.0)
                nc.vector.memset(rightT[b][DH:DH + 1, :], 1.0)

            # ---------------- main pair loop ----------------
            def build_mp4(b, sg, il):
                """[33, 512] bf16 pack for 4 i's:
                mp[c, q*128 + p] = left[b, i(sg,il,q), c] * w2[c, p]."""
                mp = mp_pool.tile([DH + 1, 512], BF16, tag="mp",
                                  name=f"mp{b}_{sg}_{il}")
                col = b * LSH + (sg * 4 + il) * 4
                lsrc = leftT[:, col:col + 4].unsqueeze(-1).to_broadcast(
                    [DH + 1, 4, PAIR])
                wsrc = w2bf.unsqueeze(1).to_broadcast([DH + 1, 4, PAIR])
                dst = mp[:, :].rearrange("c (q p) -> c q p", p=PAIR)
                nc.gpsimd.tensor_mul(out=dst, in0=wsrc, in1=lsrc)
                return mp

            # all 32 M-packs depend only on leftT; hoist them in round order
            # on the (otherwise idle) GpSimd queue so no round ever waits.
            all_mps = {}

            def build_all_mps():
                for b in range(B):
                    for q2 in range(2):
                        for s in range(2):
                            for il in range(4):
                                all_mps[(b, q2, s, il)] = build_mp4(
                                    b, q2 * 2 + s, il)

            # strict DVE/ACT drain alternation (balanced 64/64)
            DRAIN_DVE = {0: (0, 2), 1: (1, 3)}

            def pair_rounds(b):
                for q2 in range(2):
                    mps = [[all_mps[(b, q2, s, il)] for il in range(4)]
                           for s in range(2)]
                    for jc in range(4):
                        lhsT = rightT[b][:, jc * 128:(jc + 1) * 128]
                        stg = stag_pool.tile([128, 2 * 16 * PAIR], BF16,
                                             tag="stag")
                        deng = nc.gpsimd if (b, q2) == (1, 0) else nc.sync
                        for s in range(2):
                            for il in range(4):
                                pb = ps_big.tile([128, 512], F32, tag="big")
                                nc.tensor.matmul(pb, lhsT, mps[s][il],
                                                 start=True, stop=True)
                                dst = stg[:, (s * 16 + il * 4) * PAIR:
                                          (s * 16 + il * 4 + 4) * PAIR]
                                if il in DRAIN_DVE[jc % 2]:
                                    nc.vector.tensor_copy(out=dst, in_=pb)
                                else:
                                    nc.scalar.copy(out=dst, in_=pb)
                            # fire this s-half as soon as its 4 drains land
                            dst_ap = out[b, jc, q2, s, :, :, :]
                            src_ap = stg[:, s * 16 * PAIR:
                                         (s + 1) * 16 * PAIR].rearrange(
                                "j (i p) -> j i p", p=PAIR)
                            deng.dma_start(out=dst_ap, in_=src_ap)

            build_right(0)
            build_right(1)
            build_all_mps()
            pair_rounds(0)
            pair_rounds(1)

    nc.compile()
    names = ["node_full", "node_shard", "consts", "const_rows"]
    return nc, names


def _bfpack(a):
    """bf16 array [..., N] -> fp32-word-packed view [..., N//2]."""
    import ml_dtypes
    b = np.ascontiguousarray(np.asarray(a).astype(ml_dtypes.bfloat16))
    return b.view(np.float32)


def _prepare_in_maps(node, mask, ln_gamma, ln_beta, W_left, b_left, W_right,
                     b_right, W_out, b_out):
    f = np.float32
    node = np.ascontiguousarray(np.asarray(node, dtype=f))        # [B, L, D]
    mask_f = np.asarray(mask).astype(f)                           # [B, L]
    gamma = np.asarray(ln_gamma, dtype=f)
    beta = np.asarray(ln_beta, dtype=f)
    W_l = np.asarray(W_left, dtype=f)
    W_r = np.asarray(W_right, dtype=f)
    b_l = np.asarray(b_left, dtype=f)
    b_r = np.asarray(b_right, dtype=f)
    W_o = np.asarray(W_out, dtype=f)
    b_o = np.asarray(b_out, dtype=f)

    s = 1.0 / np.sqrt(np.float32(DH))
    w_left_e = np.concatenate([gamma[:, None] * W_l, (beta @ W_l)[None, :]], 0)
    w_right_e = np.concatenate([gamma[:, None] * W_r, (beta @ W_r)[None, :]],
                               0) * s
    w2 = np.concatenate([np.repeat(W_o, 2, axis=0), b_o[None, :]], 0)

    common_cst = np.zeros((128, NCONST), f)
    for dc in range(2):
        common_cst[:, CW_WL[dc]:CW_WL[dc] + 16] = \
            _bfpack(w_left_e[dc * 128:(dc + 1) * 128])
        common_cst[:, CW_WR[dc]:CW_WR[dc] + 16] = \
            _bfpack(w_right_e[dc * 128:(dc + 1) * 128])
    common_cst[0:DH + 1, CW_W2:CW_W2 + 64] = _bfpack(w2)
    common_cst[:, CW_MCF:CW_MCF + 8] = mask_f.reshape(-1, 128).T
    common_cst[0:DH, CW_BL] = b_l
    common_cst[0:DH, CW_BR] = b_r * s

    common_rows = np.zeros((1, NROWS), f)
    common_rows[0, RW_WLR:RW_WLR + 16] = _bfpack(w_left_e[D])
    common_rows[0, RW_WRR:RW_WRR + 16] = _bfpack(w_right_e[D])
    for b in range(B):
        common_rows[0, RW_MRF[b]:RW_MRF[b] + 256] = _bfpack(mask_f[b])

    node_flat = node.reshape(B * L, D)

    in_maps = []
    for c in range(NCORES):
        sl = slice(c * LSH, (c + 1) * LSH)
        shard = np.ascontiguousarray(node[:, sl, :].reshape(B * LSH, D))
        msk = mask_f[:, sl]                                       # [B, LSH]
        cstc = common_cst.copy()
        cstc[:, CW_MCS] = msk.reshape(-1)
        rowc = common_rows.copy()
        rowc[0, RW_MRS:RW_MRS + 64] = _bfpack(msk.reshape(-1))
        in_maps.append({
            "node_full": node_flat,
            "node_shard": shard,
            "consts": cstc,
            "const_rows": rowc,
        })
    return in_maps


def kernel(**inputs):
    global _COMPILED
    if _COMPILED is None:
        _COMPILED = _build_program()
    nc, names = _COMPILED
    in_maps = _prepare_in_maps(**inputs)
    res = run_bass_kernel_spmd(nc, in_maps, core_ids=list(range(NCORES)))
    full = np.empty((B, L, L, PAIR), np.float32)
    for c in range(NCORES):
        dev = res.results[c]["out"]   # [b, jc, q2, s, j, i16, p] bf16
        full[:, c * LSH:(c + 1) * LSH] = (
            dev.transpose(0, 2, 3, 5, 1, 4, 6)
               .reshape(B, LSH, L, PAIR).astype(np.float32))
    return full


if __name__ == "__main__":
    # self-test with NON-trivial gamma/beta/mask against a numpy reference
    rng = np.random.default_rng(1)
    mask = np.ones((B, L), dtype=bool)
    mask[0, 500:] = False        # exercise the mask path
    mask[1, :3] = False
    inputs = {
        "node": rng.standard_normal((B, L, D)).astype(np.float32),
        "mask": mask,
        "ln_gamma": (1.0 + 0.1 * rng.standard_normal(D)).astype(np.float32),
        "ln_beta": (0.1 * rng.standard_normal(D)).astype(np.float32),
        "W_left": (rng.standard_normal((D, DH)) / np.sqrt(D)).astype(np.float32),
        "b_left": (0.1 * rng.standard_normal(DH)).astype(np.float32),
        "W_right": (rng.standard_normal((D, DH)) / np.sqrt(D)).astype(np.float32),
        "b_right": (0.1 * rng.standard_normal(DH)).astype(np.float32),
        "W_out": (rng.standard_normal((H, PAIR)) / np.sqrt(H)).astype(np.float32),
        "b_out": (0.1 * rng.standard_normal(PAIR)).astype(np.float32),
    }

    def np_reference(node, mask, ln_gamma, ln_beta, W_left, b_left, W_right,
                     b_right, W_out, b_out):
        node = node.astype(np.float64)
        mu = node.mean(-1, keepdims=True)
        var = ((node - mu) ** 2).mean(-1, keepdims=True)
        x = (node - mu) / np.sqrt(var + LN_EPS) * ln_gamma + ln_beta
        x = x * mask[..., None]
        left = (x @ W_left + b_left).reshape(B, L, H, -1)
        right = ((x @ W_right + b_right) / np.sqrt(DH)).reshape(B, L, H, -1)
        o = np.einsum("bihk,bjhk->bijh", left, right)
        return np.einsum("bijh,hp->bijp", o, W_out) + b_out

    got = kernel(**inputs)
    exp = np_reference(**inputs)
    rel = np.abs(got - exp).max() / np.abs(exp).max()
    print("general-path rel err:", rel)
    assert rel < 1.8e-2, rel
    print("OK", got.shape, got.dtype)
